# revision 1
# baseline (speedup 1.0000x reference)
"""MoE post-processing MLP kernel for Trainium2 (8 NeuronCores).

Strategy: expert-parallel sharding. Each core is assigned one chunk of
samples routed to a single expert (K=8 experts ~= 8 cores for uniform
routing). The host gathers/permutes samples by expert and the device
runs a dense 3-layer MLP entirely in fp16 (fp32 PSUM accumulation).

Posenc on device: u36 = R^T @ fpv computes all angle/2pi rows (+phase)
on the PE; Vector forms r = round(u) with the magic-constant dual-op
tensor_scalar; a -I72 matmul accumulates -r back into the same PSUM
group so Sin reads the range-reduced angles straight from PSUM
(LUT domain [-pi,pi]).
To keep fp16 phase error small at the largest scales, the host uploads
x4~ = mod(4*view/2pi,1)-0.5 (3 extra rows/block); the m=4 row uses it
with coefficient 1 and m=8 with coefficient 2 (2*x mod 1 == (2x) mod 1).

Device layout (C=8704 samples: 8 pair-packed 512-col tiles + one
256-col tail tile, processed last so the drain is short):
  h0 = relu(W0a^T@fpv + W0s^T@s36 + b0); h1 = relu(W1^T@h0 + b1)
  y = W2^T@h1 + b2
relu0 on Scalar, relu1 on Vector, y-copy split Scalar/Vector.
DMA rings: SP carries the small fill set (tail tile + weights) and the
outputs; both bulk fpv halves stream on the GpSimd software ring (it
round-robins all 16 DMA engines; SP's hardware ring is partition-
striped and ring waits are cumulative per queue, so consumers must
never sit behind an unrelated bulk transfer).
The PE's HAM clock gate only reaches 2.4 GHz after ~5.5us of sustained
duty, so a 10-matmul zero-weight chain precedes the first real matmul
and one filler matmul pads every later tile (all-zero weights
accumulated into live PSUM groups: numerically a no-op that survives
dead-code elimination).  The tail group's yp borrows a free `up` PSUM
slot so the drain never waits on the previous group's y-copy.
"""

import numpy as np

K = 8
WID = 64
D = 32
NT = 512            # full-tile matmul moving dim (one fp32 PSUM bank)
NFULL = 8           # full tiles (1024 samples each, pair-packed)
TNT = 256           # tail-tile moving dim (512 samples)
C = NFULL * 2 * NT + 2 * TNT     # 8704 samples per core-chunk
COLS = NFULL * NT + TNT          # 4352 device columns
TWO_PI = float(2.0 * np.pi)
RB = 42             # input rows per block: feat32 pos3 view3 ones1 x4~3
CMAGIC = 12582912.0  # 1.5 * 2**23, round-to-nearest magic constant

# processing order: the four 1024-wide pair groups, then the small
# 256-col tail group last (short drain); (col0, fw) in device columns
PGROUPS = [(0, 1024), (1024, 1024), (2048, 1024), (3072, 1024),
           (4096, TNT)]

# W0 row indices (DIN=74 layout: feat 0:32, posenc(pos,2) 32:47,
# posenc(view,4) 47:74) for the identity part and the sin part.
_W0A_ROWS = list(range(32)) + [32, 33, 34] + [47, 48, 49]
_W0S_ROWS = (list(range(35, 41)) + list(range(50, 62))
             + list(range(41, 47)) + list(range(62, 74)))

_PREP = None  # compiled Bass program, built once per process
_LAST_IN_MAPS = None  # stashed for external profiling harnesses


def _build_R():
    """R' [RB, 36]: u = m*x/2pi + 0.25*phase + 0.5 per angle column.
    Rows: feat(unused) 0:32, pos 32:35, view 35:38, ones 38, x4~ 39:42.
    The m=4/m=8 view columns read the host-prereduced x4~ rows."""
    Rp = np.zeros((RB, 36), np.float32)
    col = 0
    for p in range(2):
        base_phase = 0.25 * p
        for m in (1.0, 2.0):
            for c in range(3):
                Rp[32 + c, col] = m / TWO_PI
                Rp[38, col] = base_phase
                col += 1
        for m in (1.0, 2.0):
            for c in range(3):
                Rp[35 + c, col] = m / TWO_PI
                Rp[38, col] = base_phase
                col += 1
        for coef in (1.0, 2.0):
            for c in range(3):
                Rp[39 + c, col] = coef
                # x4~ stored -0.5: u = coef*(x4+0.5)+... fold into phase
                Rp[38, col] = base_phase + 0.5 * coef
                col += 1
    return Rp


def _build_program():
    import concourse.bacc as bacc
    import concourse.mybir as mybir
    from concourse.tile import TileContext

    F32, F16 = mybir.dt.float32, mybir.dt.float16
    AF = mybir.ActivationFunctionType
    ALU = mybir.AluOpType

    nc = bacc.Bacc("TRN2", target_bir_lowering=False, debug=False,
                   num_devices=8)

    fpv_d = nc.dram_tensor("fpv", [2 * RB, COLS], F16,
                           kind="ExternalInput").ap()
    wall_d = nc.dram_tensor("wall", [128, 592], F16,
                            kind="ExternalInput").ap()
    bias_d = nc.dram_tensor("bias", [128, 3], F32, kind="ExternalInput").ap()
    y_d = nc.dram_tensor("y", [64, COLS], F16, kind="ExternalOutput").ap()

    with TileContext(nc) as tc:
        with (tc.tile_pool(name="w", bufs=1) as wp,
              tc.tile_pool(name="fp", bufs=1) as fpool,
              tc.tile_pool(name="io", bufs=4) as io,
              tc.tile_pool(name="psu", bufs=2, space="PSUM") as psu,
              tc.tile_pool(name="ps0", bufs=1, space="PSUM") as ps0,
              tc.tile_pool(name="ps1", bufs=1, space="PSUM") as ps1,
              tc.tile_pool(name="psy", bufs=1, space="PSUM") as psy):
            wall = wp.tile([128, 592], F16)
            biasw = wp.tile([128, 3], F32)
            dummy = wp.tile([128, 512], F16)
            ft_A0 = fpool.tile([2 * RB, 512], F16)
            ft_A1 = fpool.tile([2 * RB, 512], F16)
            ft_A2 = fpool.tile([2 * RB, 1024], F16)
            ft_B = fpool.tile([2 * RB, 2048], F16)
            ft_T = fpool.tile([2 * RB, TNT], F16)

            def fpv_src(c0, fw):
                if c0 >= 4096:
                    return ft_T[:, 0:fw]
                if c0 < 512:
                    return ft_A0[:, c0:c0 + fw]
                if c0 < 1024:
                    return ft_A1[:, c0 - 512:c0 - 512 + fw]
                if c0 < 2048:
                    return ft_A2[:, c0 - 1024:c0 - 1024 + fw]
                return ft_B[:, c0 - 2048:c0 - 2048 + fw]

            # DMA rings: SP = fill set + outputs; GpSimd software ring =
            # first bulk half; Scalar hardware ring = second bulk half.
            nc.gpsimd.memset(dummy[:], 0.0)
            nc.gpsimd.dma_start(out=ft_A0[:], in_=fpv_d[:, 0:512])
            nc.gpsimd.dma_start(out=ft_A1[:], in_=fpv_d[:, 512:1024])
            nc.gpsimd.dma_start(out=ft_A2[:], in_=fpv_d[:, 1024:2048])
            nc.gpsimd.dma_start(out=ft_B[:], in_=fpv_d[:, 2048:4096])
            nc.sync.dma_start(out=wall[:], in_=wall_d[:, :])
            nc.sync.dma_start(out=biasw[:], in_=bias_d[:, :])
            nc.sync.dma_start(out=ft_T[:], in_=fpv_d[:, 4096:COLS])

            Rt = wall[0:2 * RB, 0:72]
            W0at = wall[0:2 * RB, 72:200]
            W0st = wall[0:72, 200:328]
            W1t = wall[0:128, 328:456]
            W2t = wall[0:128, 456:520]
            negI = wall[0:72, 520:592]
            b0t = biasw[0:128, 0:1]
            b1t = biasw[0:128, 1:2]
            b2t = biasw[0:64, 2:3]

            for gi, (gc0, fw) in enumerate(PGROUPS):
                s36 = io.tile([72, fw], F16, name="s36")
                h0t = io.tile([128, fw], F16, name="h0t")
                h1t = io.tile([128, fw], F16, name="h1t")
                yt = io.tile([64, fw], F16, name="yt")
                h0p = ps0.tile([128, fw], F32, name="h0p")
                h1p = ps1.tile([128, fw], F32, name="h1p")
                if gi == len(PGROUPS) - 1:
                    # tail yp borrows a free `up` slot so its W2 does not
                    # wait for the previous group's y-copy via psy reuse
                    yp = psu.tile([72, NT], F32, name="up")
                else:
                    yp = psy.tile([64, fw], F32, name="yp")

                for off in range(0, fw, NT):
                    w = min(NT, fw - off)
                    js = slice(off, off + w)
                    up = psu.tile([72, NT], F32, name="up")
                    vt = io.tile([72, NT], F16, name="vt")
                    # p-state warmup / keep-warm: the HAM clock gate only
                    # reaches (and holds) 2.4 GHz under sustained PE duty,
                    # so a long zero-weight chain precedes the first real
                    # matmul and one filler pads every later tile.  All-zero
                    # weights accumulate nothing (widths vary vs dedup).
                    nd = 10 if (gi == 0 and off == 0) else 1
                    for d in range(nd):
                        nc.tensor.matmul(out=up[:, 0:NT - d],
                                         lhsT=dummy[:, 0:72],
                                         rhs=dummy[:, 0:NT - d],
                                         start=(d == 0), stop=False)
                    nc.tensor.matmul(out=up[:, 0:w], lhsT=Rt,
                                     rhs=fpv_src(gc0 + off, w),
                                     start=(nd == 0), stop=False)
                    nc.vector.tensor_scalar(out=vt[:, 0:w], in0=up[:, 0:w],
                                            scalar1=CMAGIC, scalar2=CMAGIC,
                                            op0=ALU.add, op1=ALU.subtract)
                    nc.tensor.matmul(out=up[:, 0:w], lhsT=negI,
                                     rhs=vt[:, 0:w], start=False, stop=True)
                    nc.scalar.activation(s36[:, js], up[:, 0:w], AF.Sin,
                                         bias=0.0, scale=TWO_PI)
                for off in range(0, fw, NT):
                    w = min(NT, fw - off)
                    js = slice(off, off + w)
                    nc.tensor.matmul(out=h0p[:, js], lhsT=W0at,
                                     rhs=fpv_src(gc0 + off, w),
                                     start=True, stop=False)
                    nc.tensor.matmul(out=h0p[:, js], lhsT=W0st,
                                     rhs=s36[:, js], start=False, stop=True)
                nc.scalar.activation(h0t[:], h0p[:], AF.Relu,
                                     bias=b0t, scale=1.0)
                for off in range(0, fw, NT):
                    w = min(NT, fw - off)
                    js = slice(off, off + w)
                    nc.tensor.matmul(out=h1p[:, js], lhsT=W1t,
                                     rhs=h0t[:, js], start=True, stop=True)
                nc.vector.tensor_scalar(out=h1t[:], in0=h1p[:],
                                        scalar1=b1t, scalar2=0.0,
                                        op0=ALU.add, op1=ALU.max)
                for off in range(0, fw, NT):
                    w = min(NT, fw - off)
                    js = slice(off, off + w)
                    nc.tensor.matmul(out=yp[0:64, js], lhsT=W2t,
                                     rhs=h1t[:, js], start=True, stop=True)
                ys = (fw * 62 // 100) & ~31
                nc.scalar.activation(yt[:, 0:ys], yp[0:64, 0:ys],
                                     AF.Identity, bias=b2t, scale=1.0)
                nc.vector.tensor_scalar(out=yt[:, ys:fw],
                                        in0=yp[0:64, ys:fw],
                                        scalar1=b2t, scalar2=None,
                                        op0=ALU.add)
                nc.sync.dma_start(out=y_d[:, gc0:gc0 + fw], in_=yt[:])

    nc.compile()
    return nc


def _get_program():
    global _PREP
    if _PREP is None:
        _PREP = _build_program()
    return _PREP


def _pack_weights(W0, b0, W1, b1, W2, b2):
    """Per-expert [128, 592] fp16 weight wall + [128, 3] f32 biases."""
    W0a = np.zeros((RB, 64), np.float32)
    W0a[0:38] = W0[_W0A_ROWS]
    W0s = W0[_W0S_ROWS].astype(np.float32)          # [36, 64]
    Rp = _build_R()
    wall = np.zeros((128, 592), np.float16)
    wall[0:RB, 0:36] = Rp
    wall[RB:2 * RB, 36:72] = Rp
    wall[0:RB, 72:136] = W0a
    wall[RB:2 * RB, 136:200] = W0a
    wall[0:36, 200:264] = W0s
    wall[36:72, 264:328] = W0s
    wall[0:64, 328:392] = W1
    wall[64:128, 392:456] = W1
    wall[0:64, 456:488] = W2
    wall[64:128, 488:520] = W2
    wall[0:72, 520:592] = -np.eye(72, dtype=np.float16)
    bias = np.zeros((128, 3), np.float32)
    bias[:, 0] = np.concatenate([b0, b0])
    bias[:, 1] = np.concatenate([b1, b1])
    bias[0:64, 2] = np.concatenate([b2, b2])
    return wall, bias


def _pack_cols(data, n):
    """[R, C-samples] -> [2R, COLS] pair-packed device layout."""
    R = data.shape[0]
    full = data[:, :NFULL * 2 * NT].reshape(R, NFULL, 2, NT)
    fullp = np.concatenate([full[:, :, 0], full[:, :, 1]],
                           axis=0).reshape(2 * R, NFULL * NT)
    tail = data[:, NFULL * 2 * NT:].reshape(R, 1, 2, TNT)
    tailp = np.concatenate([tail[:, :, 0], tail[:, :, 1]],
                           axis=0).reshape(2 * R, TNT)
    return np.concatenate([fullp, tailp], axis=1)


def _unpack_cols(y):
    """[64, COLS] device layout -> [32, C] sample order."""
    yf = y[:, :NFULL * NT].reshape(64, NFULL, NT)
    full = np.stack([yf[0:32], yf[32:64]], axis=2).reshape(32, NFULL * 2 * NT)
    ytl = y[:, NFULL * NT:].reshape(64, 1, TNT)
    tail = np.stack([ytl[0:32], ytl[32:64]], axis=2).reshape(32, 2 * TNT)
    return np.concatenate([full, tail], axis=1)


def kernel(idxs, positions, viewdirs, features, W0, b0, W1, b1, W2, b2):
    from concourse.bass_utils import run_bass_kernel_spmd

    N = idxs.shape[0]
    idx = idxs.reshape(-1).astype(np.int64)
    out = np.zeros((N, D), np.float32)

    # Route: list of (expert, sample-index-array) chunks of <= C samples.
    chunks = []
    for k in range(K):
        sel = np.nonzero(idx == k)[0]
        for lo in range(0, len(sel), C):
            chunks.append((k, sel[lo:lo + C]))

    walls = [_pack_weights(W0[k], b0[k], W1[k], b1[k], W2[k], b2[k])
             for k in range(K)]

    nc = _get_program()
    zero_in = None
    for inv in range(0, len(chunks), 8):
        batch = chunks[inv:inv + 8]
        in_maps = []
        for ci in range(8):
            if ci < len(batch):
                k, sel = batch[ci]
                n = len(sel)
                fpv = np.zeros((RB, C), np.float16)
                fpv[0:32, :n] = features[sel].T
                fpv[32:35, :n] = positions[sel].T
                fpv[35:38, :n] = viewdirs[sel].T
                fpv[38, :] = 1.0
                x4 = (4.0 / TWO_PI) * viewdirs[sel].astype(np.float32)
                x4 = (x4 - np.floor(x4)) - np.float32(0.5)
                fpv[39:42, :n] = x4.T
                in_maps.append({"fpv": np.ascontiguousarray(
                                    _pack_cols(fpv, n)),
                                "wall": walls[k][0],
                                "bias": walls[k][1]})
            else:
                if zero_in is None:
                    zi = np.zeros((RB, C), np.float16)
                    zi[38, :] = 1.0
                    zero_in = {"fpv": np.ascontiguousarray(
                                   _pack_cols(zi, 0)),
                               "wall": walls[0][0],
                               "bias": walls[0][1]}
                in_maps.append(zero_in)
        global _LAST_IN_MAPS
        _LAST_IN_MAPS = in_maps
        res = None
        for attempt in range(3):
            try:
                res = run_bass_kernel_spmd(nc, in_maps,
                                           core_ids=list(range(8)))
                break
            except Exception:
                if attempt == 2:
                    raise
        assert res is not None
        for ci, (k, sel) in enumerate(batch):
            y64 = np.asarray(res.results[ci]["y"], np.float32)  # [64, COLS]
            y32 = _unpack_cols(y64)
            out[sel] = y32[:, :len(sel)].T
    return out



# revision 5
# speedup vs baseline: 1.0619x; 1.0619x over previous
"""MoE post-processing MLP kernel for Trainium2 (8 NeuronCores).

Strategy: expert-parallel sharding. Each core is assigned one chunk of
samples routed to a single expert (K=8 experts ~= 8 cores for uniform
routing). The host gathers/permutes samples by expert, computes the 36
posenc sin features (np.sin is cheap host-side and more precise than
the fp16 device path), and the device runs a dense 3-layer MLP in fp16
(fp32 PSUM accumulation):

  h0 = relu(W0a^T@xa + W0s^T@s36 + b0)   xa = [feat,pos,view] 38 rows
  h1 = relu(W1^T@h0 + b1)                s36 = posenc sins, 36 rows
  y  = W2^T@h1 + b2

Device layout: pair-packed (2 samples per column; weights duplicated
block-diagonally so the full 128-partition contract dim is used).
COLS = 8*512 + 128 = 4224 columns = 8448 samples per core, sized to
the actual max per-expert count (8336) instead of a generic bound.

Processing unit is a 1024-col group (two 512-col PSUM tiles):
  PE: W0a(t0) W0a(t1) W0s(t0) W0s(t1) W1(t0) W1(t1) W2(t0) W2(t1)
  (same-weight matmuls adjacent to amortize LDWEIGHTS, which costs
  ~contract-dim cycles pipelined behind the previous matmul)
W2(t0) targets PSUM partitions 0:64 and W2(t1) partitions 64:128 of
ONE [128,512] bank via PE tile_position=(0,64), so a single full-width
tensor_scalar emits y for both tiles (the ACT/DVE fixed ~350-cycle
per-op overhead makes many small ops the enemy).
Element-wise split: relu0 per 512 on Scalar (activation Relu + b0),
relu1 per 1024 on Vector (add-b1/max-0), y_pair alternating
Scalar/Vector per group; Pool cannot read PSUM on TRN2.
PSUM: h0p 512x2bufs (2 banks) + h1p 1024x2bufs (4) + yp 512x2bufs (2).

DMA rings: SP hardware ring carries the fill set (weights + first
512-col chunk) and the y outputs; the bulk fa/fs columns stream on the
Pool software ring (round-robins all 16 DMA engines; SP's hardware
ring is partition-striped and ring waits are cumulative per queue, so
consumers must never sit behind an unrelated bulk transfer).
The PE's HAM clock gate only reaches 2.4 GHz after a few us of
sustained duty, so a zero-weight matmul chain (accumulating harmlessly
into tile 0's live PSUM group) runs while the first DMA chunk lands.
"""

import numpy as np

K = 8
WID = 64
D = 32
NT = 512            # full-tile matmul moving dim (one fp32 PSUM bank)
NFULL = 8           # full tiles (1024 samples each, pair-packed)
TNT = 128           # tail-tile moving dim (256 samples)
C = NFULL * 2 * NT + 2 * TNT     # 8448 samples per core-chunk
COLS = NFULL * NT + TNT          # 4224 device columns
NGRP = NFULL // 2   # 4 full groups of 1024 cols
YCOLS = NGRP * NT + TNT          # 2176 output dram columns

RA = 38             # xa rows: feat 32 + pos 3 + view 3
RS = 36             # s36 rows

# W0 row indices (DIN=74 layout: feat 0:32, posenc(pos,2) 32:47,
# posenc(view,4) 47:74) for the identity part and the sin part.
_W0A_ROWS = list(range(32)) + [32, 33, 34] + [47, 48, 49]
_W0S_ROWS = (list(range(35, 41)) + list(range(50, 62))
             + list(range(41, 47)) + list(range(62, 74)))

_PREP = None  # compiled Bass program, built once per process
_LAST_IN_MAPS = None  # stashed for external profiling harnesses


def _build_program():
    import concourse.bacc as bacc
    import concourse.mybir as mybir
    from concourse.tile import TileContext

    F32, F16 = mybir.dt.float32, mybir.dt.float16
    AF = mybir.ActivationFunctionType
    ALU = mybir.AluOpType

    nc = bacc.Bacc("TRN2", target_bir_lowering=False, debug=False,
                   num_devices=8)

    fpv_d = nc.dram_tensor("fpv", [2 * RA + 2 * RS, COLS], F16,
                           kind="ExternalInput").ap()
    wall_d = nc.dram_tensor("wall", [128, 448], F16,
                            kind="ExternalInput").ap()
    bias_d = nc.dram_tensor("bias", [128, 3], F32, kind="ExternalInput").ap()
    y_d = nc.dram_tensor("y", [128, YCOLS], F16, kind="ExternalOutput").ap()

    with TileContext(nc) as tc:
        with (tc.tile_pool(name="w", bufs=1) as wp,
              tc.tile_pool(name="fp", bufs=1) as fpool,
              tc.tile_pool(name="io", bufs=8) as io,
              tc.tile_pool(name="ps0", bufs=2, space="PSUM") as ps0,
              tc.tile_pool(name="ps1", bufs=2, space="PSUM") as ps1,
              tc.tile_pool(name="psy", bufs=2, space="PSUM") as psy):
            wall = wp.tile([128, 448], F16)
            biasw = wp.tile([128, 3], F32)
            dummy = wp.tile([128, NT], F16)
            fa = fpool.tile([2 * RA, COLS], F16)
            fs = fpool.tile([2 * RS, COLS], F16)

            # DMA rings: SP = fill set + outputs; Pool software ring =
            # bulk columns (round-robins all 16 DMA engines).
            nc.vector.memset(dummy[:], 0.0)
            nc.sync.dma_start(out=wall[:], in_=wall_d[:, :])
            nc.sync.dma_start(out=biasw[:], in_=bias_d[:, :])
            nc.sync.dma_start(out=fa[:, 0:NT], in_=fpv_d[0:2 * RA, 0:NT])
            nc.sync.dma_start(out=fs[:, 0:NT],
                              in_=fpv_d[2 * RA:2 * RA + 2 * RS, 0:NT])
            SPLIT = 2304
            nc.gpsimd.dma_start(out=fa[:, NT:SPLIT],
                                in_=fpv_d[0:2 * RA, NT:SPLIT])
            nc.gpsimd.dma_start(out=fs[:, NT:SPLIT],
                                in_=fpv_d[2 * RA:2 * RA + 2 * RS, NT:SPLIT])
            nc.gpsimd.dma_start(out=fa[:, SPLIT:COLS],
                                in_=fpv_d[0:2 * RA, SPLIT:COLS])
            nc.gpsimd.dma_start(out=fs[:, SPLIT:COLS],
                                in_=fpv_d[2 * RA:2 * RA + 2 * RS, SPLIT:COLS])

            W0at = wall[0:2 * RA, 0:128]
            W0st = wall[0:2 * RS, 128:256]
            W1t = wall[0:128, 256:384]
            W2t = wall[0:128, 384:448]
            b0t = biasw[0:128, 0:1]
            b1t = biasw[0:128, 1:2]
            b2t64 = biasw[0:64, 2:3]
            b2t = biasw[0:128, 2:3]

            for g in range(NGRP):
                c0, c1 = 2 * g * NT, (2 * g + 1) * NT
                h0p0 = ps0.tile([128, NT], F32, name="h0p")
                h0p1 = ps0.tile([128, NT], F32, name="h0p")
                h1p = ps1.tile([128, 2 * NT], F32, name="h1p")
                yp = psy.tile([128, NT], F32, name="yp")
                h0t0 = io.tile([128, NT], F16, name="h0t")
                h0t1 = io.tile([128, NT], F16, name="h0t")
                h1t = io.tile([128, 2 * NT], F16, name="h1t")
                yt = io.tile([128, NT], F16, name="yt")

                # p-state warmup: the HAM clock gate only reaches (and
                # holds) 2.4 GHz under sustained PE duty, so a zero-weight
                # chain runs while the first DMA chunk lands (accumulating
                # zeros into the live h0p group survives dead-code elim;
                # widths vary vs dedup).
                nd = 10 if g == 0 else 0
                for d in range(nd):
                    nc.tensor.matmul(out=h0p0[:, 0:NT - d],
                                     lhsT=dummy[:, 0:128],
                                     rhs=dummy[:, 0:NT - d],
                                     start=(d == 0), stop=False)
                nc.tensor.matmul(out=h0p0[:], lhsT=W0at,
                                 rhs=fa[:, c0:c0 + NT],
                                 start=(nd == 0), stop=False)
                nc.tensor.matmul(out=h0p1[:], lhsT=W0at,
                                 rhs=fa[:, c1:c1 + NT],
                                 start=True, stop=False)
                nc.tensor.matmul(out=h0p0[:], lhsT=W0st,
                                 rhs=fs[:, c0:c0 + NT],
                                 start=False, stop=True)
                nc.tensor.matmul(out=h0p1[:], lhsT=W0st,
                                 rhs=fs[:, c1:c1 + NT],
                                 start=False, stop=True)
                nc.scalar.activation(h0t0[:], h0p0[:], AF.Relu,
                                     bias=b0t, scale=1.0)
                nc.scalar.activation(h0t1[:], h0p1[:], AF.Relu,
                                     bias=b0t, scale=1.0)
                nc.tensor.matmul(out=h1p[:, 0:NT], lhsT=W1t, rhs=h0t0[:],
                                 start=True, stop=True)
                nc.tensor.matmul(out=h1p[:, NT:2 * NT], lhsT=W1t,
                                 rhs=h0t1[:], start=True, stop=True)
                nc.vector.tensor_scalar(out=h1t[:], in0=h1p[:],
                                        scalar1=b1t, scalar2=0.0,
                                        op0=ALU.add, op1=ALU.max)
                nc.tensor.matmul(out=yp[0:64, :], lhsT=W2t,
                                 rhs=h1t[:, 0:NT], start=True, stop=True)
                nc.tensor.matmul(out=yp[64:128, :], lhsT=W2t,
                                 rhs=h1t[:, NT:2 * NT], start=True,
                                 stop=True, tile_position=(0, 64))
                if g % 2 == 0:
                    nc.scalar.activation(yt[:], yp[:], AF.Identity,
                                         bias=b2t, scale=1.0)
                else:
                    nc.vector.tensor_scalar(out=yt[:], in0=yp[:],
                                            scalar1=b2t, scalar2=None,
                                            op0=ALU.add)
                nc.sync.dma_start(out=y_d[:, g * NT:(g + 1) * NT],
                                  in_=yt[:])

            # tail tile: 128 cols (256 samples)
            tc0 = NFULL * NT
            h0pt = ps0.tile([128, TNT], F32, name="h0p")
            h1pt = ps1.tile([128, TNT], F32, name="h1p")
            ypt = psy.tile([64, TNT], F32, name="yp")
            h0tt = io.tile([128, TNT], F16, name="h0t")
            h1tt = io.tile([128, TNT], F16, name="h1t")
            ytt = io.tile([64, TNT], F16, name="yt")
            nc.tensor.matmul(out=h0pt[:], lhsT=W0at,
                             rhs=fa[:, tc0:COLS], start=True, stop=False)
            nc.tensor.matmul(out=h0pt[:], lhsT=W0st,
                             rhs=fs[:, tc0:COLS], start=False, stop=True)
            nc.scalar.activation(h0tt[:], h0pt[:], AF.Relu,
                                 bias=b0t, scale=1.0)
            nc.tensor.matmul(out=h1pt[:], lhsT=W1t, rhs=h0tt[:],
                             start=True, stop=True)
            nc.vector.tensor_scalar(out=h1tt[:], in0=h1pt[:],
                                    scalar1=b1t, scalar2=0.0,
                                    op0=ALU.add, op1=ALU.max)
            nc.tensor.matmul(out=ypt[:], lhsT=W2t, rhs=h1tt[:],
                             start=True, stop=True)
            nc.scalar.activation(ytt[:], ypt[:], AF.Identity,
                                 bias=b2t64, scale=1.0)
            nc.sync.dma_start(out=y_d[0:64, NGRP * NT:YCOLS], in_=ytt[:])

    nc.compile()
    return nc


def _get_program():
    global _PREP
    if _PREP is None:
        _PREP = _build_program()
    return _PREP


def _pack_weights(W0, b0, W1, b1, W2, b2):
    """Per-expert [128, 448] fp16 weight wall + [128, 3] f32 biases."""
    W0a = W0[_W0A_ROWS].astype(np.float32)          # [38, 64]
    W0s = W0[_W0S_ROWS].astype(np.float32)          # [36, 64]
    wall = np.zeros((128, 448), np.float16)
    wall[0:RA, 0:64] = W0a
    wall[RA:2 * RA, 64:128] = W0a
    wall[0:RS, 128:192] = W0s
    wall[RS:2 * RS, 192:256] = W0s
    wall[0:64, 256:320] = W1
    wall[64:128, 320:384] = W1
    wall[0:64, 384:416] = W2
    wall[64:128, 416:448] = W2
    bias = np.zeros((128, 3), np.float32)
    bias[:, 0] = np.concatenate([b0, b0])
    bias[:, 1] = np.concatenate([b1, b1])
    bias[:, 2] = np.concatenate([b2, b2, b2, b2])
    return wall, bias


def _pack_cols(data, n):
    """[R, C-samples] -> [2R, COLS] pair-packed device layout."""
    R = data.shape[0]
    full = data[:, :NFULL * 2 * NT].reshape(R, NFULL, 2, NT)
    fullp = np.concatenate([full[:, :, 0], full[:, :, 1]],
                           axis=0).reshape(2 * R, NFULL * NT)
    tail = data[:, NFULL * 2 * NT:].reshape(R, 1, 2, TNT)
    tailp = np.concatenate([tail[:, :, 0], tail[:, :, 1]],
                           axis=0).reshape(2 * R, TNT)
    return np.concatenate([fullp, tailp], axis=1)


def _unpack_y(y):
    """[128, YCOLS] device layout -> [32, C] sample order.

    Full groups: y[0:64, g*NT:(g+1)*NT] is tile 2g, y[64:128, ...] is
    tile 2g+1; each [64, NT] tile pair-unpacks to [32, 2*NT] samples.
    Tail: y[0:64, NGRP*NT:] is the [64, TNT] tail tile."""
    parts = []
    for g in range(NGRP):
        blk = y[:, g * NT:(g + 1) * NT]
        for t64 in (blk[0:64], blk[64:128]):
            parts.append(np.concatenate([t64[0:32], t64[32:64]], axis=1))
    t64 = y[0:64, NGRP * NT:YCOLS]
    parts.append(np.concatenate([t64[0:32], t64[32:64]], axis=1))
    return np.concatenate(parts, axis=1)


def _s36(pos, view):
    """Posenc sin features in _W0S_ROWS order: pos sin (m=1,2), view sin
    (m=1,2,4,8), pos cos, view cos.  [n, 36] fp32."""
    sin_part = np.concatenate([pos, 2.0 * pos,
                               view, 2.0 * view, 4.0 * view, 8.0 * view],
                              axis=1).astype(np.float32)        # [n, 18]
    ang = np.concatenate([sin_part, sin_part + np.float32(0.5 * np.pi)],
                         axis=1)
    return np.sin(ang)


def kernel(idxs, positions, viewdirs, features, W0, b0, W1, b1, W2, b2):
    from concourse.bass_utils import run_bass_kernel_spmd

    N = idxs.shape[0]
    idx = idxs.reshape(-1).astype(np.int64)
    out = np.zeros((N, D), np.float32)

    # Route: list of (expert, sample-index-array) chunks of <= C samples.
    chunks = []
    for k in range(K):
        sel = np.nonzero(idx == k)[0]
        for lo in range(0, len(sel), C):
            chunks.append((k, sel[lo:lo + C]))

    walls = [_pack_weights(W0[k], b0[k], W1[k], b1[k], W2[k], b2[k])
             for k in range(K)]

    nc = _get_program()
    zero_in = None
    for inv in range(0, len(chunks), 8):
        batch = chunks[inv:inv + 8]
        in_maps = []
        for ci in range(8):
            if ci < len(batch):
                k, sel = batch[ci]
                n = len(sel)
                fpv = np.zeros((RA + RS, C), np.float16)
                fpv[0:32, :n] = features[sel].T
                fpv[32:35, :n] = positions[sel].T
                fpv[35:38, :n] = viewdirs[sel].T
                fpv[RA:RA + RS, :n] = _s36(positions[sel],
                                           viewdirs[sel]).T
                fa = _pack_cols(fpv[0:RA], n)          # [76, COLS]
                fs = _pack_cols(fpv[RA:RA + RS], n)    # [72, COLS]
                in_maps.append({"fpv": np.ascontiguousarray(
                                    np.concatenate([fa, fs], axis=0)),
                                "wall": walls[k][0],
                                "bias": walls[k][1]})
            else:
                if zero_in is None:
                    zero_in = {"fpv": np.zeros((2 * RA + 2 * RS, COLS),
                                               np.float16),
                               "wall": walls[0][0],
                               "bias": walls[0][1]}
                in_maps.append(zero_in)
        global _LAST_IN_MAPS
        _LAST_IN_MAPS = in_maps
        res = None
        for attempt in range(3):
            try:
                res = run_bass_kernel_spmd(nc, in_maps,
                                           core_ids=list(range(8)))
                break
            except Exception:
                if attempt == 2:
                    raise
        assert res is not None
        for ci, (k, sel) in enumerate(batch):
            y128 = np.asarray(res.results[ci]["y"], np.float32)
            y32 = _unpack_y(y128)
            out[sel] = y32[:, :len(sel)].T
    return out


# revision 8
# speedup vs baseline: 1.1546x; 1.0873x over previous
"""MoE post-processing MLP kernel for Trainium2 (8 NeuronCores).

Strategy: expert-parallel sharding. Each core is assigned one chunk of
samples routed to a single expert (K=8 experts ~= 8 cores for uniform
routing). The host gathers/permutes samples by expert, computes the 36
posenc sin features (np.sin is cheap host-side and more precise than
the fp16 device path), and the device runs a dense 3-layer MLP in fp16
(fp32 PSUM accumulation):

  h0 = relu(W0a^T@xa + W0s^T@s36 + b0)   xa = [feat,pos,view] 38 rows
  h1 = relu(W1^T@h0 + b1)                s36 = posenc sins, 36 rows
  y  = W2^T@h1 + b2

Device layout: pair-packed (2 samples per column; weights duplicated
block-diagonally so the full 128-partition contract dim is used).
COLS = 8*512 + 128 = 4224 columns = 8448 samples per core, sized to
the actual max per-expert count (8336) instead of a generic bound.

Processing unit is a 1024-col group (two 512-col PSUM tiles), with the
PE issue stream SOFTWARE-PIPELINED so every matmul's producers ran at
least one group earlier (no within-group PE->Scalar->PE round trips,
which stall the PE and keep the HAM clock gate from holding 2.4 GHz):
  iter g issues:  W0a/W0s(g+1) x4 | W1(g) x2 | W2(g-1) x2
W2(t0) targets PSUM partitions 0:64 and W2(t1) partitions 64:128 of
ONE [128,512] bank via PE tile_position=(0,64), so a single full-width
tensor_scalar emits y for both tiles (the ACT/DVE fixed ~350-cycle
per-op overhead makes many small ops the enemy).
Element-wise: relu0 per 512 on Scalar (activation Relu + b0), relu1
per 1024 on Vector (add-b1/max-0), y_pair alternating Scalar/Vector
per group; Pool cannot read PSUM on TRN2.
PSUM: h0p 512x2bufs (2 banks) + h1p 1024x2bufs (4) + yp 512x2bufs (2).

Input chunks are SEPARATE SBUF tiles (fa0/fa1/.., fs0/..): the Tile
framework tracks dependencies per tile, so a single [76, COLS] tile
would make the first matmul wait on the LAST bulk DMA chunk (observed
8us PE stall).  Rings: SP carries fa0/fs0 (first 512 cols) + y outs;
Scalar hwdge carries bias/wall + the 512:1536 chunks; Pool software
ring streams the remaining bulk (round-robins all 16 DMA engines).
A short zero-weight warmup chain bridges engine start to first-data
(accumulating zeros into g0's live h0p group survives DCE) and starts
the HAM duty ramp early.
"""

import numpy as np

K = 8
WID = 64
D = 32
NT = 512            # full-tile matmul moving dim (one fp32 PSUM bank)
NFULL = 8           # full tiles (1024 samples each, pair-packed)
TNT = 128           # tail-tile moving dim (256 samples)
C = NFULL * 2 * NT + 2 * TNT     # 8448 samples per core-chunk
COLS = NFULL * NT + TNT          # 4224 device columns
NGRP = NFULL // 2   # 4 full groups of 1024 cols; group NGRP = tail
YCOLS = NGRP * NT + TNT          # 2176 output dram columns

RA = 38             # xa rows: feat 32 + pos 3 + view 3
RS = 36             # s36 rows

# input chunk boundaries (512-aligned; separate SBUF tiles per chunk)
CHUNKS = [(0, 512), (512, 1536), (1536, 2560), (2560, 3584),
          (3584, COLS)]

# W0 row indices (DIN=74 layout: feat 0:32, posenc(pos,2) 32:47,
# posenc(view,4) 47:74) for the identity part and the sin part.
_W0A_ROWS = list(range(32)) + [32, 33, 34] + [47, 48, 49]
_W0S_ROWS = (list(range(35, 41)) + list(range(50, 62))
             + list(range(41, 47)) + list(range(62, 74)))

_PREP = None  # compiled Bass program, built once per process
_LAST_IN_MAPS = None  # stashed for external profiling harnesses


def _build_program():
    import concourse.bacc as bacc
    import concourse.mybir as mybir
    from concourse.tile import TileContext

    F32, F16 = mybir.dt.float32, mybir.dt.float16
    AF = mybir.ActivationFunctionType
    ALU = mybir.AluOpType

    nc = bacc.Bacc("TRN2", target_bir_lowering=False, debug=False,
                   num_devices=8)

    fpv_d = nc.dram_tensor("fpv", [2 * RA + 2 * RS, COLS], F16,
                           kind="ExternalInput").ap()
    wall_d = nc.dram_tensor("wall", [128, 448], F16,
                            kind="ExternalInput").ap()
    bias_d = nc.dram_tensor("bias", [128, 3], F32, kind="ExternalInput").ap()
    y_d = nc.dram_tensor("y", [128, YCOLS], F16, kind="ExternalOutput").ap()

    with TileContext(nc) as tc:
        with (tc.tile_pool(name="w", bufs=1) as wp,
              tc.tile_pool(name="fp", bufs=1) as fpool,
              tc.tile_pool(name="io", bufs=8) as io,
              tc.tile_pool(name="ps0", bufs=2, space="PSUM") as ps0,
              tc.tile_pool(name="ps1", bufs=2, space="PSUM") as ps1,
              tc.tile_pool(name="psy", bufs=2, space="PSUM") as psy):
            wall = wp.tile([128, 448], F16)
            biasw = wp.tile([128, 3], F32)
            dummy = wp.tile([128, NT], F16)
            fat = [fpool.tile([2 * RA, c1 - c0], F16, name=f"fa{i}")
                   for i, (c0, c1) in enumerate(CHUNKS)]
            fst = [fpool.tile([2 * RS, c1 - c0], F16, name=f"fs{i}")
                   for i, (c0, c1) in enumerate(CHUNKS)]

            def fsrc(tiles, c0, w):
                for (ck0, ck1), t in zip(CHUNKS, tiles):
                    if ck0 <= c0 and c0 + w <= ck1:
                        return t[:, c0 - ck0:c0 - ck0 + w]
                raise AssertionError(f"no chunk covers {c0}+{w}")

            # DMA rings: SP = first chunk + outputs; Scalar hwdge =
            # weights + second chunk; Pool software ring = bulk.
            nc.vector.memset(dummy[:], 0.0)
            nc.sync.dma_start(out=fat[0][:], in_=fpv_d[0:2 * RA, 0:512])
            nc.sync.dma_start(out=fst[0][:],
                              in_=fpv_d[2 * RA:2 * RA + 2 * RS, 0:512])
            nc.scalar.dma_start(out=wall[:], in_=wall_d[:, :])
            nc.scalar.dma_start(out=biasw[:], in_=bias_d[:, :])
            nc.scalar.dma_start(out=fat[1][:],
                                in_=fpv_d[0:2 * RA, 512:1536])
            nc.scalar.dma_start(out=fst[1][:],
                                in_=fpv_d[2 * RA:2 * RA + 2 * RS, 512:1536])
            for i in (2, 3, 4):
                c0, c1 = CHUNKS[i]
                nc.gpsimd.dma_start(out=fat[i][:], in_=fpv_d[0:2 * RA, c0:c1])
                nc.gpsimd.dma_start(
                    out=fst[i][:],
                    in_=fpv_d[2 * RA:2 * RA + 2 * RS, c0:c1])

            W0at = wall[0:2 * RA, 0:128]
            W0st = wall[0:2 * RS, 128:256]
            W1t = wall[0:128, 256:384]
            W2t = wall[0:128, 384:448]
            b0t = biasw[0:128, 0:1]
            b1t = biasw[0:128, 1:2]
            b2t64 = biasw[0:64, 2:3]
            b2t = biasw[0:128, 2:3]

            # per-group state (group NGRP is the 128-col tail, 1 tile)
            H0P, H0T, H1P, H1T, YP, YT = {}, {}, {}, {}, {}, {}

            def gw(g):
                return TNT if g == NGRP else NT

            def gtiles(g):
                return 1 if g == NGRP else 2

            def issue_W0(g, warm=0):
                w = gw(g)
                H0P[g] = [ps0.tile([128, w], F32, name="h0p")
                          for _ in range(gtiles(g))]
                for d in range(warm):
                    nc.tensor.matmul(out=H0P[g][0][:, 0:w - d],
                                     lhsT=dummy[:, 0:128],
                                     rhs=dummy[:, 0:w - d],
                                     start=(d == 0), stop=False)
                for t in range(gtiles(g)):
                    c0 = 2 * g * NT + t * NT if g < NGRP else NFULL * NT
                    nc.tensor.matmul(out=H0P[g][t][:], lhsT=W0at,
                                     rhs=fsrc(fat, c0, w),
                                     start=(warm == 0 or t > 0), stop=False)
                for t in range(gtiles(g)):
                    c0 = 2 * g * NT + t * NT if g < NGRP else NFULL * NT
                    nc.tensor.matmul(out=H0P[g][t][:], lhsT=W0st,
                                     rhs=fsrc(fst, c0, w),
                                     start=False, stop=True)

            def issue_relu0(g):
                w = gw(g)
                H0T[g] = [io.tile([128, w], F16, name="h0t")
                          for _ in range(gtiles(g))]
                for t in range(gtiles(g)):
                    nc.scalar.activation(H0T[g][t][:], H0P[g][t][:],
                                         AF.Relu, bias=b0t, scale=1.0)

            def issue_W1(g):
                w = gw(g)
                n = gtiles(g)
                H1P[g] = ps1.tile([128, n * w], F32, name="h1p")
                for t in range(n):
                    nc.tensor.matmul(out=H1P[g][:, t * w:(t + 1) * w],
                                     lhsT=W1t, rhs=H0T[g][t][:],
                                     start=True, stop=True)

            def issue_relu1(g):
                w = gw(g) * gtiles(g)
                H1T[g] = io.tile([128, w], F16, name="h1t")
                nc.vector.tensor_scalar(out=H1T[g][:], in0=H1P[g][:],
                                        scalar1=b1t, scalar2=0.0,
                                        op0=ALU.add, op1=ALU.max)

            def issue_W2(g):
                w = gw(g)
                if g == NGRP:
                    YP[g] = psy.tile([64, w], F32, name="yp")
                    nc.tensor.matmul(out=YP[g][:], lhsT=W2t,
                                     rhs=H1T[g][:], start=True, stop=True)
                else:
                    YP[g] = psy.tile([128, w], F32, name="yp")
                    nc.tensor.matmul(out=YP[g][0:64, :], lhsT=W2t,
                                     rhs=H1T[g][:, 0:w],
                                     start=True, stop=True)
                    nc.tensor.matmul(out=YP[g][64:128, :], lhsT=W2t,
                                     rhs=H1T[g][:, w:2 * w], start=True,
                                     stop=True, tile_position=(0, 64))

            def issue_y(g):
                w = gw(g)
                if g == NGRP:
                    YT[g] = io.tile([64, w], F16, name="yt")
                    nc.scalar.activation(YT[g][:], YP[g][:], AF.Identity,
                                         bias=b2t64, scale=1.0)
                elif g % 2 == 0:
                    YT[g] = io.tile([128, w], F16, name="yt")
                    nc.scalar.activation(YT[g][:], YP[g][:], AF.Identity,
                                         bias=b2t, scale=1.0)
                else:
                    YT[g] = io.tile([128, w], F16, name="yt")
                    nc.vector.tensor_scalar(out=YT[g][:], in0=YP[g][:],
                                            scalar1=b2t, scalar2=None,
                                            op0=ALU.add)

            def issue_dma(g):
                if g == NGRP:
                    nc.sync.dma_start(out=y_d[0:64, NGRP * NT:YCOLS],
                                      in_=YT[g][:])
                else:
                    nc.sync.dma_start(out=y_d[:, g * NT:(g + 1) * NT],
                                      in_=YT[g][:])

            # software-pipelined schedule
            issue_W0(0, warm=4)
            issue_relu0(0)
            for g in range(NGRP + 1):
                if g + 1 <= NGRP:
                    issue_W0(g + 1)
                    issue_relu0(g + 1)
                issue_W1(g)
                issue_relu1(g)
                if g - 1 >= 0:
                    issue_W2(g - 1)
                    issue_y(g - 1)
                    issue_dma(g - 1)
            issue_W2(NGRP)
            issue_y(NGRP)
            issue_dma(NGRP)

    nc.compile()
    return nc


def _get_program():
    global _PREP
    if _PREP is None:
        _PREP = _build_program()
    return _PREP


def _pack_weights(W0, b0, W1, b1, W2, b2):
    """Per-expert [128, 448] fp16 weight wall + [128, 3] f32 biases."""
    W0a = W0[_W0A_ROWS].astype(np.float32)          # [38, 64]
    W0s = W0[_W0S_ROWS].astype(np.float32)          # [36, 64]
    wall = np.zeros((128, 448), np.float16)
    wall[0:RA, 0:64] = W0a
    wall[RA:2 * RA, 64:128] = W0a
    wall[0:RS, 128:192] = W0s
    wall[RS:2 * RS, 192:256] = W0s
    wall[0:64, 256:320] = W1
    wall[64:128, 320:384] = W1
    wall[0:64, 384:416] = W2
    wall[64:128, 416:448] = W2
    bias = np.zeros((128, 3), np.float32)
    bias[:, 0] = np.concatenate([b0, b0])
    bias[:, 1] = np.concatenate([b1, b1])
    bias[:, 2] = np.concatenate([b2, b2, b2, b2])
    return wall, bias


def _pack_cols(data, n):
    """[R, C-samples] -> [2R, COLS] pair-packed device layout."""
    R = data.shape[0]
    full = data[:, :NFULL * 2 * NT].reshape(R, NFULL, 2, NT)
    fullp = np.concatenate([full[:, :, 0], full[:, :, 1]],
                           axis=0).reshape(2 * R, NFULL * NT)
    tail = data[:, NFULL * 2 * NT:].reshape(R, 1, 2, TNT)
    tailp = np.concatenate([tail[:, :, 0], tail[:, :, 1]],
                           axis=0).reshape(2 * R, TNT)
    return np.concatenate([fullp, tailp], axis=1)


def _unpack_y(y):
    """[128, YCOLS] device layout -> [32, C] sample order.

    Full groups: y[0:64, g*NT:(g+1)*NT] is tile 2g, y[64:128, ...] is
    tile 2g+1; each [64, NT] tile holds samples [top 0:NT, bottom
    NT:2*NT].  Tail: y[0:64, NGRP*NT:] is the [64, TNT] tail tile."""
    parts = []
    for g in range(NGRP):
        blk = y[:, g * NT:(g + 1) * NT]
        for t64 in (blk[0:64], blk[64:128]):
            parts.append(np.concatenate([t64[0:32], t64[32:64]], axis=1))
    t64 = y[0:64, NGRP * NT:YCOLS]
    parts.append(np.concatenate([t64[0:32], t64[32:64]], axis=1))
    return np.concatenate(parts, axis=1)


def _s36(pos, view):
    """Posenc sin features in _W0S_ROWS order: pos sin (m=1,2), view sin
    (m=1,2,4,8), pos cos, view cos.  [n, 36] fp32."""
    sin_part = np.concatenate([pos, 2.0 * pos,
                               view, 2.0 * view, 4.0 * view, 8.0 * view],
                              axis=1).astype(np.float32)        # [n, 18]
    ang = np.concatenate([sin_part, sin_part + np.float32(0.5 * np.pi)],
                         axis=1)
    return np.sin(ang)


def kernel(idxs, positions, viewdirs, features, W0, b0, W1, b1, W2, b2):
    from concourse.bass_utils import run_bass_kernel_spmd

    N = idxs.shape[0]
    idx = idxs.reshape(-1).astype(np.int64)
    out = np.zeros((N, D), np.float32)

    # Route: list of (expert, sample-index-array) chunks of <= C samples.
    chunks = []
    for k in range(K):
        sel = np.nonzero(idx == k)[0]
        for lo in range(0, len(sel), C):
            chunks.append((k, sel[lo:lo + C]))

    walls = [_pack_weights(W0[k], b0[k], W1[k], b1[k], W2[k], b2[k])
             for k in range(K)]

    nc = _get_program()
    zero_in = None
    for inv in range(0, len(chunks), 8):
        batch = chunks[inv:inv + 8]
        in_maps = []
        for ci in range(8):
            if ci < len(batch):
                k, sel = batch[ci]
                n = len(sel)
                fpv = np.zeros((RA + RS, C), np.float16)
                fpv[0:32, :n] = features[sel].T
                fpv[32:35, :n] = positions[sel].T
                fpv[35:38, :n] = viewdirs[sel].T
                fpv[RA:RA + RS, :n] = _s36(positions[sel],
                                           viewdirs[sel]).T
                fa = _pack_cols(fpv[0:RA], n)          # [76, COLS]
                fs = _pack_cols(fpv[RA:RA + RS], n)    # [72, COLS]
                in_maps.append({"fpv": np.ascontiguousarray(
                                    np.concatenate([fa, fs], axis=0)),
                                "wall": walls[k][0],
                                "bias": walls[k][1]})
            else:
                if zero_in is None:
                    zero_in = {"fpv": np.zeros((2 * RA + 2 * RS, COLS),
                                               np.float16),
                               "wall": walls[0][0],
                               "bias": walls[0][1]}
                in_maps.append(zero_in)
        global _LAST_IN_MAPS
        _LAST_IN_MAPS = in_maps
        res = None
        for attempt in range(3):
            try:
                res = run_bass_kernel_spmd(nc, in_maps,
                                           core_ids=list(range(8)))
                break
            except Exception:
                if attempt == 2:
                    raise
        assert res is not None
        for ci, (k, sel) in enumerate(batch):
            y128 = np.asarray(res.results[ci]["y"], np.float32)
            y32 = _unpack_y(y128)
            out[sel] = y32[:, :len(sel)].T
    return out


# revision 11
# speedup vs baseline: 1.1917x; 1.0322x over previous
"""MoE post-processing MLP kernel for Trainium2 (8 NeuronCores).

Strategy: expert-parallel sharding. Each core is assigned one chunk of
samples routed to a single expert (K=8 experts ~= 8 cores for uniform
routing). The host gathers/permutes samples by expert, computes the 36
posenc sin features (np.sin is cheap host-side and more precise than
the fp16 device path), and the device runs a dense 3-layer MLP in fp16
(fp32 PSUM accumulation):

  h0 = relu(W0a^T@xa + W0s^T@s36 + b0)   xa = [feat,pos,view] 38 rows
  h1 = relu(W1^T@h0 + b1)                s36 = posenc sins, 36 rows
  y  = W2^T@h1 + b2

Device layout: pair-packed (2 samples per column; weights duplicated
block-diagonally so the full 128-partition contract dim is used).
COLS = 8*512 + 128 = 4224 columns = 8448 samples per core, sized to
the actual max per-expert count (8336) instead of a generic bound.

Processing unit is a 1024-col group (two 512-col PSUM tiles), with the
PE issue stream SOFTWARE-PIPELINED so every matmul's producers ran at
least one group earlier (no within-group PE->Scalar->PE round trips,
which stall the PE and keep the HAM clock gate from holding 2.4 GHz):
  iter g issues:  W0a/W0s(g+1) x4 | W1(g) x2 | W2(g-1) x2
W2(t0) targets PSUM partitions 0:64 and W2(t1) partitions 64:128 of
ONE [128,512] bank via PE tile_position=(0,64), so a single full-width
tensor_scalar emits y for both tiles (the ACT/DVE fixed ~350-cycle
per-op overhead makes many small ops the enemy).
Element-wise: relu0 per 512 on Scalar (activation Relu + b0), relu1
per 1024 on Vector (add-b1/max-0), y_pair alternating Scalar/Vector
per group; Pool cannot read PSUM on TRN2.
PSUM: h0p 512x2bufs (2 banks) + h1p 1024x2bufs (4) + yp 512x2bufs (2).

Input chunks are SEPARATE SBUF tiles (fa0/fa1/.., fs0/..): the Tile
framework tracks dependencies per tile, so a single [76, COLS] tile
would make the first matmul wait on the LAST bulk DMA chunk (observed
8us PE stall).  Rings: SP carries fa0/fs0 (first 512 cols) + y outs;
Scalar hwdge carries bias/wall + the 512:1536 chunks; Pool software
ring streams the remaining bulk (round-robins all 16 DMA engines).
A short zero-weight warmup chain bridges engine start to first-data
(accumulating zeros into g0's live h0p group survives DCE) and starts
the HAM duty ramp early.
"""

import numpy as np

K = 8
WID = 64
D = 32
NT = 512            # full-tile matmul moving dim (one fp32 PSUM bank)
NFULL = 8           # full tiles (1024 samples each, pair-packed)
TNT = 128           # tail-tile moving dim (256 samples)
C = NFULL * 2 * NT + 2 * TNT     # 8448 samples per core-chunk
COLS = NFULL * NT + TNT          # 4224 device columns
NGRP = NFULL // 2   # 4 full groups of 1024 cols; group NGRP = tail
YCOLS = NGRP * NT + TNT          # 2176 output dram columns

RA = 38             # xa rows: feat 32 + pos 3 + view 3
RS = 36             # s36 rows

# input chunk boundaries (group-aligned; separate SBUF tiles per chunk)
CHUNKS = [(0, 1024), (1024, 2048), (2048, 3072), (3072, 4096),
          (4096, COLS)]

# W0 row indices (DIN=74 layout: feat 0:32, posenc(pos,2) 32:47,
# posenc(view,4) 47:74) for the identity part and the sin part.
_W0A_ROWS = list(range(32)) + [32, 33, 34] + [47, 48, 49]
_W0S_ROWS = (list(range(35, 41)) + list(range(50, 62))
             + list(range(41, 47)) + list(range(62, 74)))

_PREP = None  # compiled Bass program, built once per process
_LAST_IN_MAPS = None  # stashed for external profiling harnesses


def _build_program():
    import concourse.bacc as bacc
    import concourse.mybir as mybir
    from concourse.tile import TileContext

    F32, F16 = mybir.dt.float32, mybir.dt.float16
    AF = mybir.ActivationFunctionType
    ALU = mybir.AluOpType

    nc = bacc.Bacc("TRN2", target_bir_lowering=False, debug=False,
                   num_devices=8)

    fpv_d = nc.dram_tensor("fpv", [2 * RA + 2 * RS, COLS], F16,
                           kind="ExternalInput").ap()
    wall_d = nc.dram_tensor("wall", [128, 448], F16,
                            kind="ExternalInput").ap()
    bias_d = nc.dram_tensor("bias", [128, 3], F32, kind="ExternalInput").ap()
    y_d = nc.dram_tensor("y", [128, YCOLS], F16, kind="ExternalOutput").ap()

    with TileContext(nc) as tc:
        with (tc.tile_pool(name="w", bufs=1) as wp,
              tc.tile_pool(name="fp", bufs=1) as fpool,
              tc.tile_pool(name="io", bufs=12) as io,
              tc.tile_pool(name="ps0", bufs=2, space="PSUM") as ps0,
              tc.tile_pool(name="ps1", bufs=2, space="PSUM") as ps1,
              tc.tile_pool(name="psy", bufs=2, space="PSUM") as psy):
            wall = wp.tile([128, 448], F16)
            biasw = wp.tile([128, 3], F32)
            dummy = wp.tile([128, NT], F16)
            fat = [fpool.tile([2 * RA, c1 - c0], F16, name=f"fa{i}")
                   for i, (c0, c1) in enumerate(CHUNKS)]
            fst = [fpool.tile([2 * RS, c1 - c0], F16, name=f"fs{i}")
                   for i, (c0, c1) in enumerate(CHUNKS)]

            def fsrc(tiles, c0, w):
                for (ck0, ck1), t in zip(CHUNKS, tiles):
                    if ck0 <= c0 and c0 + w <= ck1:
                        return t[:, c0 - ck0:c0 - ck0 + w]
                raise AssertionError(f"no chunk covers {c0}+{w}")

            # DMA rings: SP = first group's chunk + outputs; Scalar
            # hwdge = weights/bias; Pool software ring = the remaining
            # bulk, dispatched in strict consumption order (DMA engines
            # are shared across rings, so service order ~ dispatch
            # order; out-of-order bulk starves the early tiles).
            nc.vector.memset(dummy[:], 0.0)
            nc.sync.dma_start(out=fat[0][:], in_=fpv_d[0:2 * RA, 0:1024])
            nc.sync.dma_start(out=fst[0][:],
                              in_=fpv_d[2 * RA:2 * RA + 2 * RS, 0:1024])
            nc.scalar.dma_start(out=wall[:], in_=wall_d[:, :])
            nc.scalar.dma_start(out=biasw[:], in_=bias_d[:, :])
            for i in (1, 2, 3, 4):
                c0, c1 = CHUNKS[i]
                nc.gpsimd.dma_start(out=fat[i][:], in_=fpv_d[0:2 * RA, c0:c1])
                nc.gpsimd.dma_start(
                    out=fst[i][:],
                    in_=fpv_d[2 * RA:2 * RA + 2 * RS, c0:c1])

            W0at = wall[0:2 * RA, 0:128]
            W0st = wall[0:2 * RS, 128:256]
            W1t = wall[0:128, 256:384]
            W2t = wall[0:128, 384:448]
            b0t = biasw[0:128, 0:1]
            b1t = biasw[0:128, 1:2]
            b2t64 = biasw[0:64, 2:3]
            b2t = biasw[0:128, 2:3]

            # per-group state (group NGRP is the 128-col tail, 1 tile)
            H0P, H0T, H1P, H1T, YP, YT = {}, {}, {}, {}, {}, {}

            def gw(g):
                return TNT if g == NGRP else NT

            def gtiles(g):
                return 1 if g == NGRP else 2

            def issue_W0(g, warm=0):
                w = gw(g)
                H0P[g] = [ps0.tile([128, w], F32, name="h0p")
                          for _ in range(gtiles(g))]
                for d in range(warm):
                    nc.tensor.matmul(out=H0P[g][0][:, 0:w - d],
                                     lhsT=dummy[:, 0:128],
                                     rhs=dummy[:, 0:w - d],
                                     start=(d == 0), stop=False)
                for t in range(gtiles(g)):
                    c0 = 2 * g * NT + t * NT if g < NGRP else NFULL * NT
                    nc.tensor.matmul(out=H0P[g][t][:], lhsT=W0at,
                                     rhs=fsrc(fat, c0, w),
                                     start=(warm == 0 or t > 0), stop=False)
                for t in range(gtiles(g)):
                    c0 = 2 * g * NT + t * NT if g < NGRP else NFULL * NT
                    nc.tensor.matmul(out=H0P[g][t][:], lhsT=W0st,
                                     rhs=fsrc(fst, c0, w),
                                     start=False, stop=True)

            def issue_relu0(g):
                w = gw(g)
                H0T[g] = [io.tile([128, w], F16, name="h0t")
                          for _ in range(gtiles(g))]
                for t in range(gtiles(g)):
                    nc.scalar.activation(H0T[g][t][:], H0P[g][t][:],
                                         AF.Relu, bias=b0t, scale=1.0)

            def issue_W1(g):
                w = gw(g)
                n = gtiles(g)
                H1P[g] = ps1.tile([128, n * w], F32, name="h1p")
                for t in range(n):
                    nc.tensor.matmul(out=H1P[g][:, t * w:(t + 1) * w],
                                     lhsT=W1t, rhs=H0T[g][t][:],
                                     start=True, stop=True)

            def issue_relu1(g):
                w = gw(g) * gtiles(g)
                H1T[g] = io.tile([128, w], F16, name="h1t")
                nc.vector.tensor_scalar(out=H1T[g][:], in0=H1P[g][:],
                                        scalar1=b1t, scalar2=0.0,
                                        op0=ALU.add, op1=ALU.max)

            def issue_W2(g):
                w = gw(g)
                if g == NGRP:
                    YP[g] = psy.tile([64, w], F32, name="yp")
                    nc.tensor.matmul(out=YP[g][:], lhsT=W2t,
                                     rhs=H1T[g][:], start=True, stop=True)
                else:
                    YP[g] = psy.tile([128, w], F32, name="yp")
                    nc.tensor.matmul(out=YP[g][0:64, :], lhsT=W2t,
                                     rhs=H1T[g][:, 0:w],
                                     start=True, stop=True)
                    nc.tensor.matmul(out=YP[g][64:128, :], lhsT=W2t,
                                     rhs=H1T[g][:, w:2 * w], start=True,
                                     stop=True, tile_position=(0, 64))

            def issue_y(g):
                w = gw(g)
                if g == NGRP:
                    YT[g] = io.tile([64, w], F16, name="yt")
                    nc.scalar.activation(YT[g][:], YP[g][:], AF.Identity,
                                         bias=b2t64, scale=1.0)
                elif g % 2 == 0:
                    YT[g] = io.tile([128, w], F16, name="yt")
                    nc.scalar.activation(YT[g][:], YP[g][:], AF.Identity,
                                         bias=b2t, scale=1.0)
                else:
                    YT[g] = io.tile([128, w], F16, name="yt")
                    nc.vector.tensor_scalar(out=YT[g][:], in0=YP[g][:],
                                            scalar1=b2t, scalar2=None,
                                            op0=ALU.add)

            def issue_dma(g):
                if g == NGRP:
                    nc.sync.dma_start(out=y_d[0:64, NGRP * NT:YCOLS],
                                      in_=YT[g][:])
                else:
                    nc.sync.dma_start(out=y_d[:, g * NT:(g + 1) * NT],
                                      in_=YT[g][:])

            # software-pipelined schedule
            issue_W0(0, warm=6)
            issue_relu0(0)
            for g in range(NGRP + 1):
                if g + 1 <= NGRP:
                    issue_W0(g + 1)
                    issue_relu0(g + 1)
                issue_W1(g)
                issue_relu1(g)
                if g - 1 >= 0:
                    issue_W2(g - 1)
                    issue_y(g - 1)
                    issue_dma(g - 1)
            issue_W2(NGRP)
            issue_y(NGRP)
            issue_dma(NGRP)

    nc.compile()
    return nc


def _get_program():
    global _PREP
    if _PREP is None:
        _PREP = _build_program()
    return _PREP


def _pack_weights(W0, b0, W1, b1, W2, b2):
    """Per-expert [128, 448] fp16 weight wall + [128, 3] f32 biases."""
    W0a = W0[_W0A_ROWS].astype(np.float32)          # [38, 64]
    W0s = W0[_W0S_ROWS].astype(np.float32)          # [36, 64]
    wall = np.zeros((128, 448), np.float16)
    wall[0:RA, 0:64] = W0a
    wall[RA:2 * RA, 64:128] = W0a
    wall[0:RS, 128:192] = W0s
    wall[RS:2 * RS, 192:256] = W0s
    wall[0:64, 256:320] = W1
    wall[64:128, 320:384] = W1
    wall[0:64, 384:416] = W2
    wall[64:128, 416:448] = W2
    bias = np.zeros((128, 3), np.float32)
    bias[:, 0] = np.concatenate([b0, b0])
    bias[:, 1] = np.concatenate([b1, b1])
    bias[:, 2] = np.concatenate([b2, b2, b2, b2])
    return wall, bias


def _pack_cols(data, n):
    """[R, C-samples] -> [2R, COLS] pair-packed device layout."""
    R = data.shape[0]
    full = data[:, :NFULL * 2 * NT].reshape(R, NFULL, 2, NT)
    fullp = np.concatenate([full[:, :, 0], full[:, :, 1]],
                           axis=0).reshape(2 * R, NFULL * NT)
    tail = data[:, NFULL * 2 * NT:].reshape(R, 1, 2, TNT)
    tailp = np.concatenate([tail[:, :, 0], tail[:, :, 1]],
                           axis=0).reshape(2 * R, TNT)
    return np.concatenate([fullp, tailp], axis=1)


def _unpack_y(y):
    """[128, YCOLS] device layout -> [32, C] sample order.

    Full groups: y[0:64, g*NT:(g+1)*NT] is tile 2g, y[64:128, ...] is
    tile 2g+1; each [64, NT] tile holds samples [top 0:NT, bottom
    NT:2*NT].  Tail: y[0:64, NGRP*NT:] is the [64, TNT] tail tile."""
    parts = []
    for g in range(NGRP):
        blk = y[:, g * NT:(g + 1) * NT]
        for t64 in (blk[0:64], blk[64:128]):
            parts.append(np.concatenate([t64[0:32], t64[32:64]], axis=1))
    t64 = y[0:64, NGRP * NT:YCOLS]
    parts.append(np.concatenate([t64[0:32], t64[32:64]], axis=1))
    return np.concatenate(parts, axis=1)


def _s36(pos, view):
    """Posenc sin features in _W0S_ROWS order: pos sin (m=1,2), view sin
    (m=1,2,4,8), pos cos, view cos.  [n, 36] fp32."""
    sin_part = np.concatenate([pos, 2.0 * pos,
                               view, 2.0 * view, 4.0 * view, 8.0 * view],
                              axis=1).astype(np.float32)        # [n, 18]
    ang = np.concatenate([sin_part, sin_part + np.float32(0.5 * np.pi)],
                         axis=1)
    return np.sin(ang)


def kernel(idxs, positions, viewdirs, features, W0, b0, W1, b1, W2, b2):
    from concourse.bass_utils import run_bass_kernel_spmd

    N = idxs.shape[0]
    idx = idxs.reshape(-1).astype(np.int64)
    out = np.zeros((N, D), np.float32)

    # Route: list of (expert, sample-index-array) chunks of <= C samples.
    chunks = []
    for k in range(K):
        sel = np.nonzero(idx == k)[0]
        for lo in range(0, len(sel), C):
            chunks.append((k, sel[lo:lo + C]))

    walls = [_pack_weights(W0[k], b0[k], W1[k], b1[k], W2[k], b2[k])
             for k in range(K)]

    nc = _get_program()
    zero_in = None
    for inv in range(0, len(chunks), 8):
        batch = chunks[inv:inv + 8]
        in_maps = []
        for ci in range(8):
            if ci < len(batch):
                k, sel = batch[ci]
                n = len(sel)
                fpv = np.zeros((RA + RS, C), np.float16)
                fpv[0:32, :n] = features[sel].T
                fpv[32:35, :n] = positions[sel].T
                fpv[35:38, :n] = viewdirs[sel].T
                fpv[RA:RA + RS, :n] = _s36(positions[sel],
                                           viewdirs[sel]).T
                fa = _pack_cols(fpv[0:RA], n)          # [76, COLS]
                fs = _pack_cols(fpv[RA:RA + RS], n)    # [72, COLS]
                in_maps.append({"fpv": np.ascontiguousarray(
                                    np.concatenate([fa, fs], axis=0)),
                                "wall": walls[k][0],
                                "bias": walls[k][1]})
            else:
                if zero_in is None:
                    zero_in = {"fpv": np.zeros((2 * RA + 2 * RS, COLS),
                                               np.float16),
                               "wall": walls[0][0],
                               "bias": walls[0][1]}
                in_maps.append(zero_in)
        global _LAST_IN_MAPS
        _LAST_IN_MAPS = in_maps
        res = None
        for attempt in range(3):
            try:
                res = run_bass_kernel_spmd(nc, in_maps,
                                           core_ids=list(range(8)))
                break
            except Exception:
                if attempt == 2:
                    raise
        assert res is not None
        for ci, (k, sel) in enumerate(batch):
            y128 = np.asarray(res.results[ci]["y"], np.float32)
            y32 = _unpack_y(y128)
            out[sel] = y32[:, :len(sel)].T
    return out


# revision 14
# speedup vs baseline: 1.2488x; 1.0479x over previous
"""MoE post-processing MLP kernel for Trainium2 (8 NeuronCores).

Strategy: expert-parallel sharding. Each core is assigned one chunk of
samples routed to a single expert (K=8 experts ~= 8 cores for uniform
routing). The host gathers/permutes samples by expert, computes the 36
posenc sin features (np.sin is cheap host-side and more precise than
the fp16 device path), and the device runs a dense 3-layer MLP in fp16
(fp32 PSUM accumulation):

  h0 = relu(W0a^T@xa + W0s^T@s36 + b0)   xa = [feat,pos,view] 38 rows
  h1 = relu(W1^T@h0 + b1)                s36 = posenc sins, 36 rows
  y  = W2^T@h1 + b2

Device layout: pair-packed (2 samples per column; weights duplicated
block-diagonally so the full 128-partition contract dim is used).
COLS = 8*512 + 128 = 4224 columns = 8448 samples per core, sized to
the actual max per-expert count (8336) instead of a generic bound.

Processing unit is a 1024-col group (two 512-col PSUM tiles), with the
PE issue stream SOFTWARE-PIPELINED so every matmul's producers ran at
least one group earlier (no within-group PE->Scalar->PE round trips,
which stall the PE and keep the HAM clock gate from holding 2.4 GHz):
  iter g issues:  W0a/W0s(g+1) x4 | W1(g) x2 | W2(g-1) x2
W2(t0) targets PSUM partitions 0:64 and W2(t1) partitions 64:128 of
ONE [128,512] bank via PE tile_position=(0,64), so a single full-width
tensor_scalar emits y for both tiles (the ACT/DVE fixed ~350-cycle
per-op overhead makes many small ops the enemy).
Element-wise: relu0 per 512 on Scalar (activation Relu + b0), relu1
per 1024 on Vector (add-b1/max-0), y_pair alternating Scalar/Vector
per group; Pool cannot read PSUM on TRN2.
PSUM: h0p 512x2bufs (2 banks) + h1p 1024x2bufs (4) + yp 512x2bufs (2).

Input chunks are SEPARATE SBUF tiles (fa0/fa1/.., fs0/..): the Tile
framework tracks dependencies per tile, so a single [76, COLS] tile
would make the first matmul wait on the LAST bulk DMA chunk (observed
8us PE stall).  Rings: SP carries fa0/fs0 (first 512 cols) + y outs;
Scalar hwdge carries bias/wall + the 512:1536 chunks; Pool software
ring streams the remaining bulk (round-robins all 16 DMA engines).
A short zero-weight warmup chain bridges engine start to first-data
(accumulating zeros into g0's live h0p group survives DCE) and starts
the HAM duty ramp early.
"""

import numpy as np

K = 8
WID = 64
D = 32
NT = 512            # full-tile matmul moving dim (one fp32 PSUM bank)
NFULL = 8           # full tiles (1024 samples each, pair-packed)
TNT = 128           # tail-tile moving dim (256 samples)
C = NFULL * 2 * NT + 2 * TNT     # 8448 samples per core-chunk
COLS = NFULL * NT + TNT          # 4224 device columns
NGRP = NFULL // 2   # 4 full groups of 1024 cols; group NGRP = tail
YCOLS = NGRP * NT + TNT          # 2176 output dram columns

RA = 38             # xa rows: feat 32 + pos 3 + view 3
RS = 36             # s36 rows

# input chunk boundaries (512-aligned; separate SBUF tiles per chunk)
CHUNKS = [(0, 512), (512, 1024), (1024, 2048), (2048, 3072),
          (3072, 4096), (4096, COLS)]

# W0 row indices (DIN=74 layout: feat 0:32, posenc(pos,2) 32:47,
# posenc(view,4) 47:74) for the identity part and the sin part.
_W0A_ROWS = list(range(32)) + [32, 33, 34] + [47, 48, 49]
_W0S_ROWS = (list(range(35, 41)) + list(range(50, 62))
             + list(range(41, 47)) + list(range(62, 74)))

_PREP = None  # compiled Bass program, built once per process
_LAST_IN_MAPS = None  # stashed for external profiling harnesses


def _build_program():
    import concourse.bacc as bacc
    import concourse.mybir as mybir
    from concourse.tile import TileContext

    F32, F16 = mybir.dt.float32, mybir.dt.float16
    AF = mybir.ActivationFunctionType
    ALU = mybir.AluOpType

    nc = bacc.Bacc("TRN2", target_bir_lowering=False, debug=False,
                   num_devices=8)

    fpv_d = nc.dram_tensor("fpv", [2 * RA + 2 * RS, COLS], F16,
                           kind="ExternalInput").ap()
    wall_d = nc.dram_tensor("wall", [128, 448], F16,
                            kind="ExternalInput").ap()
    bias_d = nc.dram_tensor("bias", [128, 3], F32, kind="ExternalInput").ap()
    y_d = nc.dram_tensor("y", [128, YCOLS], F16, kind="ExternalOutput").ap()

    with TileContext(nc) as tc:
        with (tc.tile_pool(name="w", bufs=1) as wp,
              tc.tile_pool(name="fp", bufs=1) as fpool,
              tc.tile_pool(name="io", bufs=12) as io,
              tc.tile_pool(name="ps0", bufs=2, space="PSUM") as ps0,
              tc.tile_pool(name="ps1", bufs=2, space="PSUM") as ps1,
              tc.tile_pool(name="psy", bufs=2, space="PSUM") as psy):
            wall = wp.tile([128, 448], F16)
            biasw = wp.tile([128, 3], F32)
            dummy = wp.tile([128, NT], F16)
            fat = [fpool.tile([2 * RA, c1 - c0], F16, name=f"fa{i}")
                   for i, (c0, c1) in enumerate(CHUNKS)]
            fst = [fpool.tile([2 * RS, c1 - c0], F16, name=f"fs{i}")
                   for i, (c0, c1) in enumerate(CHUNKS)]

            def fsrc(tiles, c0, w):
                for (ck0, ck1), t in zip(CHUNKS, tiles):
                    if ck0 <= c0 and c0 + w <= ck1:
                        return t[:, c0 - ck0:c0 - ck0 + w]
                raise AssertionError(f"no chunk covers {c0}+{w}")

            # DMA rings: SP = first group's chunk + outputs; Scalar
            # hwdge = weights/bias; Pool software ring = the remaining
            # bulk, dispatched in strict consumption order (DMA engines
            # are shared across rings, so service order ~ dispatch
            # order; out-of-order bulk starves the early tiles).
            nc.vector.memset(dummy[:], 0.0)
            nc.sync.dma_start(out=wall[:], in_=wall_d[:, :])
            nc.sync.dma_start(out=biasw[:], in_=bias_d[:, :])
            for i in range(len(CHUNKS)):
                c0, c1 = CHUNKS[i]
                nc.gpsimd.dma_start(out=fat[i][:], in_=fpv_d[0:2 * RA, c0:c1])
                nc.gpsimd.dma_start(
                    out=fst[i][:],
                    in_=fpv_d[2 * RA:2 * RA + 2 * RS, c0:c1])

            W0at = wall[0:2 * RA, 0:128]
            W0st = wall[0:2 * RS, 128:256]
            W1t = wall[0:128, 256:384]
            W2t = wall[0:128, 384:448]
            b0t = biasw[0:128, 0:1]
            b1t = biasw[0:128, 1:2]
            b2t64 = biasw[0:64, 2:3]
            b2t = biasw[0:128, 2:3]

            # per-group state (group NGRP is the 128-col tail, 1 tile)
            H0P, H0T, H1P, H1T, YP, YT = {}, {}, {}, {}, {}, {}

            def gw(g):
                return TNT if g == NGRP else NT

            def gtiles(g):
                return 1 if g == NGRP else 2

            def issue_W0(g, warm=0):
                w = gw(g)
                H0P[g] = [ps0.tile([128, w], F32, name="h0p")
                          for _ in range(gtiles(g))]
                for d in range(warm):
                    nc.tensor.matmul(out=H0P[g][0][:, 0:w - d],
                                     lhsT=dummy[:, 0:128],
                                     rhs=dummy[:, 0:w - d],
                                     start=(d == 0), stop=False)
                for t in range(gtiles(g)):
                    c0 = 2 * g * NT + t * NT if g < NGRP else NFULL * NT
                    nc.tensor.matmul(out=H0P[g][t][:], lhsT=W0at,
                                     rhs=fsrc(fat, c0, w),
                                     start=(warm == 0 or t > 0), stop=False)
                for t in range(gtiles(g)):
                    c0 = 2 * g * NT + t * NT if g < NGRP else NFULL * NT
                    nc.tensor.matmul(out=H0P[g][t][:], lhsT=W0st,
                                     rhs=fsrc(fst, c0, w),
                                     start=False, stop=True)

            def issue_relu0(g):
                w = gw(g)
                H0T[g] = [io.tile([128, w], F16, name="h0t")
                          for _ in range(gtiles(g))]
                for t in range(gtiles(g)):
                    nc.scalar.activation(H0T[g][t][:], H0P[g][t][:],
                                         AF.Relu, bias=b0t, scale=1.0)

            def issue_W1(g):
                w = gw(g)
                n = gtiles(g)
                H1P[g] = ps1.tile([128, n * w], F32, name="h1p")
                for t in range(n):
                    nc.tensor.matmul(out=H1P[g][:, t * w:(t + 1) * w],
                                     lhsT=W1t, rhs=H0T[g][t][:],
                                     start=True, stop=True)

            def issue_relu1(g):
                w = gw(g) * gtiles(g)
                H1T[g] = io.tile([128, w], F16, name="h1t")
                nc.vector.tensor_scalar(out=H1T[g][:], in0=H1P[g][:],
                                        scalar1=b1t, scalar2=0.0,
                                        op0=ALU.add, op1=ALU.max)

            def issue_W2(g):
                w = gw(g)
                if g == NGRP:
                    YP[g] = psy.tile([64, w], F32, name="yp")
                    nc.tensor.matmul(out=YP[g][:], lhsT=W2t,
                                     rhs=H1T[g][:], start=True, stop=True)
                else:
                    YP[g] = psy.tile([128, w], F32, name="yp")
                    nc.tensor.matmul(out=YP[g][0:64, :], lhsT=W2t,
                                     rhs=H1T[g][:, 0:w],
                                     start=True, stop=True)
                    nc.tensor.matmul(out=YP[g][64:128, :], lhsT=W2t,
                                     rhs=H1T[g][:, w:2 * w], start=True,
                                     stop=True, tile_position=(0, 64))

            def issue_y(g):
                w = gw(g)
                if g == NGRP:
                    YT[g] = io.tile([64, w], F16, name="yt")
                    nc.scalar.activation(YT[g][:], YP[g][:], AF.Identity,
                                         bias=b2t64, scale=1.0)
                elif g % 2 == 0:
                    YT[g] = io.tile([128, w], F16, name="yt")
                    nc.scalar.activation(YT[g][:], YP[g][:], AF.Identity,
                                         bias=b2t, scale=1.0)
                else:
                    YT[g] = io.tile([128, w], F16, name="yt")
                    nc.vector.tensor_scalar(out=YT[g][:], in0=YP[g][:],
                                            scalar1=b2t, scalar2=None,
                                            op0=ALU.add)

            def issue_dma(g):
                if g == NGRP:
                    nc.sync.dma_start(out=y_d[0:64, NGRP * NT:YCOLS],
                                      in_=YT[g][:])
                else:
                    nc.sync.dma_start(out=y_d[:, g * NT:(g + 1) * NT],
                                      in_=YT[g][:])

            # software-pipelined schedule
            issue_W0(0, warm=3)
            issue_relu0(0)
            for g in range(NGRP + 1):
                if g + 1 <= NGRP:
                    issue_W0(g + 1)
                    issue_relu0(g + 1)
                issue_W1(g)
                issue_relu1(g)
                if g - 1 >= 0:
                    issue_W2(g - 1)
                    issue_y(g - 1)
                    issue_dma(g - 1)
            issue_W2(NGRP)
            issue_y(NGRP)
            issue_dma(NGRP)

    nc.compile()
    return nc


def _get_program():
    global _PREP
    if _PREP is None:
        _PREP = _build_program()
    return _PREP


def _pack_weights(W0, b0, W1, b1, W2, b2):
    """Per-expert [128, 448] fp16 weight wall + [128, 3] f32 biases."""
    W0a = W0[_W0A_ROWS].astype(np.float32)          # [38, 64]
    W0s = W0[_W0S_ROWS].astype(np.float32)          # [36, 64]
    wall = np.zeros((128, 448), np.float16)
    wall[0:RA, 0:64] = W0a
    wall[RA:2 * RA, 64:128] = W0a
    wall[0:RS, 128:192] = W0s
    wall[RS:2 * RS, 192:256] = W0s
    wall[0:64, 256:320] = W1
    wall[64:128, 320:384] = W1
    wall[0:64, 384:416] = W2
    wall[64:128, 416:448] = W2
    bias = np.zeros((128, 3), np.float32)
    bias[:, 0] = np.concatenate([b0, b0])
    bias[:, 1] = np.concatenate([b1, b1])
    bias[:, 2] = np.concatenate([b2, b2, b2, b2])
    return wall, bias


def _pack_cols(data, n):
    """[R, C-samples] -> [2R, COLS] pair-packed device layout."""
    R = data.shape[0]
    full = data[:, :NFULL * 2 * NT].reshape(R, NFULL, 2, NT)
    fullp = np.concatenate([full[:, :, 0], full[:, :, 1]],
                           axis=0).reshape(2 * R, NFULL * NT)
    tail = data[:, NFULL * 2 * NT:].reshape(R, 1, 2, TNT)
    tailp = np.concatenate([tail[:, :, 0], tail[:, :, 1]],
                           axis=0).reshape(2 * R, TNT)
    return np.concatenate([fullp, tailp], axis=1)


def _unpack_y(y):
    """[128, YCOLS] device layout -> [32, C] sample order.

    Full groups: y[0:64, g*NT:(g+1)*NT] is tile 2g, y[64:128, ...] is
    tile 2g+1; each [64, NT] tile holds samples [top 0:NT, bottom
    NT:2*NT].  Tail: y[0:64, NGRP*NT:] is the [64, TNT] tail tile."""
    parts = []
    for g in range(NGRP):
        blk = y[:, g * NT:(g + 1) * NT]
        for t64 in (blk[0:64], blk[64:128]):
            parts.append(np.concatenate([t64[0:32], t64[32:64]], axis=1))
    t64 = y[0:64, NGRP * NT:YCOLS]
    parts.append(np.concatenate([t64[0:32], t64[32:64]], axis=1))
    return np.concatenate(parts, axis=1)


def _s36(pos, view):
    """Posenc sin features in _W0S_ROWS order: pos sin (m=1,2), view sin
    (m=1,2,4,8), pos cos, view cos.  [n, 36] fp32."""
    sin_part = np.concatenate([pos, 2.0 * pos,
                               view, 2.0 * view, 4.0 * view, 8.0 * view],
                              axis=1).astype(np.float32)        # [n, 18]
    ang = np.concatenate([sin_part, sin_part + np.float32(0.5 * np.pi)],
                         axis=1)
    return np.sin(ang)


def kernel(idxs, positions, viewdirs, features, W0, b0, W1, b1, W2, b2):
    from concourse.bass_utils import run_bass_kernel_spmd

    N = idxs.shape[0]
    idx = idxs.reshape(-1).astype(np.int64)
    out = np.zeros((N, D), np.float32)

    # Route: list of (expert, sample-index-array) chunks of <= C samples.
    chunks = []
    for k in range(K):
        sel = np.nonzero(idx == k)[0]
        for lo in range(0, len(sel), C):
            chunks.append((k, sel[lo:lo + C]))

    walls = [_pack_weights(W0[k], b0[k], W1[k], b1[k], W2[k], b2[k])
             for k in range(K)]

    nc = _get_program()
    zero_in = None
    for inv in range(0, len(chunks), 8):
        batch = chunks[inv:inv + 8]
        in_maps = []
        for ci in range(8):
            if ci < len(batch):
                k, sel = batch[ci]
                n = len(sel)
                fpv = np.zeros((RA + RS, C), np.float16)
                fpv[0:32, :n] = features[sel].T
                fpv[32:35, :n] = positions[sel].T
                fpv[35:38, :n] = viewdirs[sel].T
                fpv[RA:RA + RS, :n] = _s36(positions[sel],
                                           viewdirs[sel]).T
                fa = _pack_cols(fpv[0:RA], n)          # [76, COLS]
                fs = _pack_cols(fpv[RA:RA + RS], n)    # [72, COLS]
                in_maps.append({"fpv": np.ascontiguousarray(
                                    np.concatenate([fa, fs], axis=0)),
                                "wall": walls[k][0],
                                "bias": walls[k][1]})
            else:
                if zero_in is None:
                    zero_in = {"fpv": np.zeros((2 * RA + 2 * RS, COLS),
                                               np.float16),
                               "wall": walls[0][0],
                               "bias": walls[0][1]}
                in_maps.append(zero_in)
        global _LAST_IN_MAPS
        _LAST_IN_MAPS = in_maps
        res = None
        for attempt in range(3):
            try:
                res = run_bass_kernel_spmd(nc, in_maps,
                                           core_ids=list(range(8)))
                break
            except Exception:
                if attempt == 2:
                    raise
        assert res is not None
        for ci, (k, sel) in enumerate(batch):
            y128 = np.asarray(res.results[ci]["y"], np.float32)
            y32 = _unpack_y(y128)
            out[sel] = y32[:, :len(sel)].T
    return out


# revision 16
# speedup vs baseline: 1.2691x; 1.0162x over previous
"""MoE post-processing MLP kernel for Trainium2 (8 NeuronCores).

Strategy: expert-parallel sharding. Each core is assigned one chunk of
samples routed to a single expert (K=8 experts ~= 8 cores for uniform
routing). The host gathers/permutes samples by expert, computes the 36
posenc sin features (np.sin is cheap host-side and more precise than
the fp16 device path), and the device runs a dense 3-layer MLP in fp16
(fp32 PSUM accumulation):

  h0 = relu(W0a^T@xa + W0s^T@s36 + b0)   xa = [feat,pos,view] 38 rows
  h1 = relu(W1^T@h0 + b1)                s36 = posenc sins, 36 rows
  y  = W2^T@h1 + b2

Device layout: pair-packed (2 samples per column; weights duplicated
block-diagonally so the full 128-partition contract dim is used).
COLS = 8*512 + 128 = 4224 columns = 8448 samples per core, sized to
the actual max per-expert count (8336) instead of a generic bound.

Processing unit is a 1024-col group (two 512-col PSUM tiles), with the
PE issue stream SOFTWARE-PIPELINED so every matmul's producers ran at
least one group earlier (no within-group PE->Scalar->PE round trips,
which stall the PE and keep the HAM clock gate from holding 2.4 GHz):
  iter g issues:  W0a/W0s(g+1) x4 | W1(g) x2 | W2(g-1) x2
W2(t0) targets PSUM partitions 0:64 and W2(t1) partitions 64:128 of
ONE [128,512] bank via PE tile_position=(0,64), so a single full-width
tensor_scalar emits y for both tiles (the ACT/DVE fixed ~350-cycle
per-op overhead makes many small ops the enemy).
Element-wise: relu0 per 512 on Scalar (activation Relu + b0), relu1
per 1024 on Vector (add-b1/max-0), y_pair alternating Scalar/Vector
per group; Pool cannot read PSUM on TRN2.
PSUM: h0p 512x2bufs (2 banks) + h1p 1024x2bufs (4) + yp 512x2bufs (2).

Input chunks are SEPARATE SBUF tiles (fa0/fa1/.., fs0/..): the Tile
framework tracks dependencies per tile, so a single [76, COLS] tile
would make the first matmul wait on the LAST bulk DMA chunk (observed
8us PE stall).  Rings: SP carries fa0/fs0 (first 512 cols) + y outs;
Scalar hwdge carries bias/wall + the 512:1536 chunks; Pool software
ring streams the remaining bulk (round-robins all 16 DMA engines).
A short zero-weight warmup chain bridges engine start to first-data
(accumulating zeros into g0's live h0p group survives DCE) and starts
the HAM duty ramp early.
"""

import numpy as np

K = 8
WID = 64
D = 32
NT = 512            # full-tile matmul moving dim (one fp32 PSUM bank)
NFULL = 8           # full tiles (1024 samples each, pair-packed)
TNT = 128           # tail-tile moving dim (256 samples)
C = NFULL * 2 * NT + 2 * TNT     # 8448 samples per core-chunk
COLS = NFULL * NT + TNT          # 4224 device columns
NGRP = NFULL // 2   # 4 full groups of 1024 cols; group NGRP = tail
YCOLS = NGRP * NT + TNT          # 2176 output dram columns

RA = 38             # xa rows: feat 32 + pos 3 + view 3
RS = 36             # s36 rows

# input chunk boundaries (512-aligned; separate SBUF tiles per chunk)
CHUNKS = [(0, 512), (512, 1024), (1024, 2048), (2048, 3072),
          (3072, 4096), (4096, COLS)]

# W0 row indices (DIN=74 layout: feat 0:32, posenc(pos,2) 32:47,
# posenc(view,4) 47:74) for the identity part and the sin part.
_W0A_ROWS = list(range(32)) + [32, 33, 34] + [47, 48, 49]
_W0S_ROWS = (list(range(35, 41)) + list(range(50, 62))
             + list(range(41, 47)) + list(range(62, 74)))

_PREP = None  # compiled Bass program, built once per process
_LAST_IN_MAPS = None  # stashed for external profiling harnesses


def _build_program():
    import concourse.bacc as bacc
    import concourse.mybir as mybir
    from concourse.tile import TileContext

    F32, F16 = mybir.dt.float32, mybir.dt.float16
    AF = mybir.ActivationFunctionType
    ALU = mybir.AluOpType

    nc = bacc.Bacc("TRN2", target_bir_lowering=False, debug=False,
                   num_devices=8)

    fpv_d = nc.dram_tensor("fpv", [2 * RA + 2 * RS, COLS], F16,
                           kind="ExternalInput").ap()
    wall_d = nc.dram_tensor("wall", [128, 448], F16,
                            kind="ExternalInput").ap()
    bias_d = nc.dram_tensor("bias", [128, 3], F32, kind="ExternalInput").ap()
    y_d = nc.dram_tensor("y", [128, YCOLS], F16, kind="ExternalOutput").ap()

    with TileContext(nc) as tc:
        with (tc.tile_pool(name="w", bufs=1) as wp,
              tc.tile_pool(name="fp", bufs=1) as fpool,
              tc.tile_pool(name="io", bufs=12) as io,
              tc.tile_pool(name="ps0", bufs=2, space="PSUM") as ps0,
              tc.tile_pool(name="ps1", bufs=2, space="PSUM") as ps1,
              tc.tile_pool(name="psy", bufs=2, space="PSUM") as psy):
            wall = wp.tile([128, 448], F16)
            biasw = wp.tile([128, 3], F32)
            dummy = wp.tile([128, NT], F16)
            fat = [fpool.tile([2 * RA, c1 - c0], F16, name=f"fa{i}")
                   for i, (c0, c1) in enumerate(CHUNKS)]
            fst = [fpool.tile([2 * RS, c1 - c0], F16, name=f"fs{i}")
                   for i, (c0, c1) in enumerate(CHUNKS)]

            def fsrc(tiles, c0, w):
                for (ck0, ck1), t in zip(CHUNKS, tiles):
                    if ck0 <= c0 and c0 + w <= ck1:
                        return t[:, c0 - ck0:c0 - ck0 + w]
                raise AssertionError(f"no chunk covers {c0}+{w}")

            # DMA rings: SP = first group's chunk + outputs; Scalar
            # hwdge = weights/bias; Pool software ring = the remaining
            # bulk, dispatched in strict consumption order (DMA engines
            # are shared across rings, so service order ~ dispatch
            # order; out-of-order bulk starves the early tiles).
            nc.vector.memset(dummy[:], 0.0)
            nc.sync.dma_start(out=wall[:], in_=wall_d[:, :])
            nc.sync.dma_start(out=biasw[:], in_=bias_d[:, :])
            # fa chunks on the Pool software ring, fs chunks on the
            # Scalar hwdge ring — two queues transfer concurrently
            # (one queue serializes its own transfers at ~75 GB/s),
            # each in strict consumption order.
            for i in range(len(CHUNKS)):
                c0, c1 = CHUNKS[i]
                nc.gpsimd.dma_start(out=fat[i][:], in_=fpv_d[0:2 * RA, c0:c1])
                nc.scalar.dma_start(
                    out=fst[i][:],
                    in_=fpv_d[2 * RA:2 * RA + 2 * RS, c0:c1])

            W0at = wall[0:2 * RA, 0:128]
            W0st = wall[0:2 * RS, 128:256]
            W1t = wall[0:128, 256:384]
            W2t = wall[0:128, 384:448]
            b0t = biasw[0:128, 0:1]
            b1t = biasw[0:128, 1:2]
            b2t64 = biasw[0:64, 2:3]
            b2t = biasw[0:128, 2:3]

            # per-group state (group NGRP is the 128-col tail, 1 tile)
            H0P, H0T, H1P, H1T, YP, YT = {}, {}, {}, {}, {}, {}

            def gw(g):
                return TNT if g == NGRP else NT

            def gtiles(g):
                return 1 if g == NGRP else 2

            def issue_W0(g, warm=0):
                w = gw(g)
                H0P[g] = [ps0.tile([128, w], F32, name="h0p")
                          for _ in range(gtiles(g))]
                for d in range(warm):
                    nc.tensor.matmul(out=H0P[g][0][:, 0:w - d],
                                     lhsT=dummy[:, 0:128],
                                     rhs=dummy[:, 0:w - d],
                                     start=(d == 0), stop=False)
                for t in range(gtiles(g)):
                    c0 = 2 * g * NT + t * NT if g < NGRP else NFULL * NT
                    nc.tensor.matmul(out=H0P[g][t][:], lhsT=W0at,
                                     rhs=fsrc(fat, c0, w),
                                     start=(warm == 0 or t > 0), stop=False)
                for t in range(gtiles(g)):
                    c0 = 2 * g * NT + t * NT if g < NGRP else NFULL * NT
                    nc.tensor.matmul(out=H0P[g][t][:], lhsT=W0st,
                                     rhs=fsrc(fst, c0, w),
                                     start=False, stop=True)

            def issue_relu0(g):
                w = gw(g)
                H0T[g] = [io.tile([128, w], F16, name="h0t")
                          for _ in range(gtiles(g))]
                for t in range(gtiles(g)):
                    nc.scalar.activation(H0T[g][t][:], H0P[g][t][:],
                                         AF.Relu, bias=b0t, scale=1.0)

            def issue_W1(g):
                w = gw(g)
                n = gtiles(g)
                H1P[g] = ps1.tile([128, n * w], F32, name="h1p")
                for t in range(n):
                    nc.tensor.matmul(out=H1P[g][:, t * w:(t + 1) * w],
                                     lhsT=W1t, rhs=H0T[g][t][:],
                                     start=True, stop=True)

            def issue_relu1(g):
                w = gw(g) * gtiles(g)
                H1T[g] = io.tile([128, w], F16, name="h1t")
                nc.vector.tensor_scalar(out=H1T[g][:], in0=H1P[g][:],
                                        scalar1=b1t, scalar2=0.0,
                                        op0=ALU.add, op1=ALU.max)

            def issue_W2(g):
                w = gw(g)
                if g == NGRP:
                    YP[g] = psy.tile([64, w], F32, name="yp")
                    nc.tensor.matmul(out=YP[g][:], lhsT=W2t,
                                     rhs=H1T[g][:], start=True, stop=True)
                else:
                    YP[g] = psy.tile([128, w], F32, name="yp")
                    nc.tensor.matmul(out=YP[g][0:64, :], lhsT=W2t,
                                     rhs=H1T[g][:, 0:w],
                                     start=True, stop=True)
                    nc.tensor.matmul(out=YP[g][64:128, :], lhsT=W2t,
                                     rhs=H1T[g][:, w:2 * w], start=True,
                                     stop=True, tile_position=(0, 64))

            def issue_y(g):
                w = gw(g)
                if g == NGRP:
                    YT[g] = io.tile([64, w], F16, name="yt")
                    nc.scalar.activation(YT[g][:], YP[g][:], AF.Identity,
                                         bias=b2t64, scale=1.0)
                elif g % 2 == 0:
                    YT[g] = io.tile([128, w], F16, name="yt")
                    nc.scalar.activation(YT[g][:], YP[g][:], AF.Identity,
                                         bias=b2t, scale=1.0)
                else:
                    YT[g] = io.tile([128, w], F16, name="yt")
                    nc.vector.tensor_scalar(out=YT[g][:], in0=YP[g][:],
                                            scalar1=b2t, scalar2=None,
                                            op0=ALU.add)

            def issue_dma(g):
                if g == NGRP:
                    nc.sync.dma_start(out=y_d[0:64, NGRP * NT:YCOLS],
                                      in_=YT[g][:])
                else:
                    nc.sync.dma_start(out=y_d[:, g * NT:(g + 1) * NT],
                                      in_=YT[g][:])

            # software-pipelined schedule
            issue_W0(0, warm=4)
            issue_relu0(0)
            for g in range(NGRP + 1):
                if g + 1 <= NGRP:
                    issue_W0(g + 1)
                    issue_relu0(g + 1)
                issue_W1(g)
                issue_relu1(g)
                if g - 1 >= 0:
                    issue_W2(g - 1)
                    issue_y(g - 1)
                    issue_dma(g - 1)
            issue_W2(NGRP)
            issue_y(NGRP)
            issue_dma(NGRP)

    nc.compile()
    return nc


def _get_program():
    global _PREP
    if _PREP is None:
        _PREP = _build_program()
    return _PREP


def _pack_weights(W0, b0, W1, b1, W2, b2):
    """Per-expert [128, 448] fp16 weight wall + [128, 3] f32 biases."""
    W0a = W0[_W0A_ROWS].astype(np.float32)          # [38, 64]
    W0s = W0[_W0S_ROWS].astype(np.float32)          # [36, 64]
    wall = np.zeros((128, 448), np.float16)
    wall[0:RA, 0:64] = W0a
    wall[RA:2 * RA, 64:128] = W0a
    wall[0:RS, 128:192] = W0s
    wall[RS:2 * RS, 192:256] = W0s
    wall[0:64, 256:320] = W1
    wall[64:128, 320:384] = W1
    wall[0:64, 384:416] = W2
    wall[64:128, 416:448] = W2
    bias = np.zeros((128, 3), np.float32)
    bias[:, 0] = np.concatenate([b0, b0])
    bias[:, 1] = np.concatenate([b1, b1])
    bias[:, 2] = np.concatenate([b2, b2, b2, b2])
    return wall, bias


def _pack_cols(data, n):
    """[R, C-samples] -> [2R, COLS] pair-packed device layout."""
    R = data.shape[0]
    full = data[:, :NFULL * 2 * NT].reshape(R, NFULL, 2, NT)
    fullp = np.concatenate([full[:, :, 0], full[:, :, 1]],
                           axis=0).reshape(2 * R, NFULL * NT)
    tail = data[:, NFULL * 2 * NT:].reshape(R, 1, 2, TNT)
    tailp = np.concatenate([tail[:, :, 0], tail[:, :, 1]],
                           axis=0).reshape(2 * R, TNT)
    return np.concatenate([fullp, tailp], axis=1)


def _unpack_y(y):
    """[128, YCOLS] device layout -> [32, C] sample order.

    Full groups: y[0:64, g*NT:(g+1)*NT] is tile 2g, y[64:128, ...] is
    tile 2g+1; each [64, NT] tile holds samples [top 0:NT, bottom
    NT:2*NT].  Tail: y[0:64, NGRP*NT:] is the [64, TNT] tail tile."""
    parts = []
    for g in range(NGRP):
        blk = y[:, g * NT:(g + 1) * NT]
        for t64 in (blk[0:64], blk[64:128]):
            parts.append(np.concatenate([t64[0:32], t64[32:64]], axis=1))
    t64 = y[0:64, NGRP * NT:YCOLS]
    parts.append(np.concatenate([t64[0:32], t64[32:64]], axis=1))
    return np.concatenate(parts, axis=1)


def _s36(pos, view):
    """Posenc sin features in _W0S_ROWS order: pos sin (m=1,2), view sin
    (m=1,2,4,8), pos cos, view cos.  [n, 36] fp32."""
    sin_part = np.concatenate([pos, 2.0 * pos,
                               view, 2.0 * view, 4.0 * view, 8.0 * view],
                              axis=1).astype(np.float32)        # [n, 18]
    ang = np.concatenate([sin_part, sin_part + np.float32(0.5 * np.pi)],
                         axis=1)
    return np.sin(ang)


def kernel(idxs, positions, viewdirs, features, W0, b0, W1, b1, W2, b2):
    from concourse.bass_utils import run_bass_kernel_spmd

    N = idxs.shape[0]
    idx = idxs.reshape(-1).astype(np.int64)
    out = np.zeros((N, D), np.float32)

    # Route: list of (expert, sample-index-array) chunks of <= C samples.
    chunks = []
    for k in range(K):
        sel = np.nonzero(idx == k)[0]
        for lo in range(0, len(sel), C):
            chunks.append((k, sel[lo:lo + C]))

    walls = [_pack_weights(W0[k], b0[k], W1[k], b1[k], W2[k], b2[k])
             for k in range(K)]

    nc = _get_program()
    zero_in = None
    for inv in range(0, len(chunks), 8):
        batch = chunks[inv:inv + 8]
        in_maps = []
        for ci in range(8):
            if ci < len(batch):
                k, sel = batch[ci]
                n = len(sel)
                fpv = np.zeros((RA + RS, C), np.float16)
                fpv[0:32, :n] = features[sel].T
                fpv[32:35, :n] = positions[sel].T
                fpv[35:38, :n] = viewdirs[sel].T
                fpv[RA:RA + RS, :n] = _s36(positions[sel],
                                           viewdirs[sel]).T
                fa = _pack_cols(fpv[0:RA], n)          # [76, COLS]
                fs = _pack_cols(fpv[RA:RA + RS], n)    # [72, COLS]
                in_maps.append({"fpv": np.ascontiguousarray(
                                    np.concatenate([fa, fs], axis=0)),
                                "wall": walls[k][0],
                                "bias": walls[k][1]})
            else:
                if zero_in is None:
                    zero_in = {"fpv": np.zeros((2 * RA + 2 * RS, COLS),
                                               np.float16),
                               "wall": walls[0][0],
                               "bias": walls[0][1]}
                in_maps.append(zero_in)
        global _LAST_IN_MAPS
        _LAST_IN_MAPS = in_maps
        res = None
        for attempt in range(3):
            try:
                res = run_bass_kernel_spmd(nc, in_maps,
                                           core_ids=list(range(8)))
                break
            except Exception:
                if attempt == 2:
                    raise
        assert res is not None
        for ci, (k, sel) in enumerate(batch):
            y128 = np.asarray(res.results[ci]["y"], np.float32)
            y32 = _unpack_y(y128)
            out[sel] = y32[:, :len(sel)].T
    return out
